# revision 1
# baseline (speedup 1.0000x reference)
"""CausalSelfAttention TRN2 kernel: LN + QKV + causal attention + out_proj.

Sharding: 8 cores = 4 batches x 2 head-groups (8 heads each). Each core
computes its batch's LayerNorm, QKV for its heads, causal softmax attention,
and a partial out-projection over its heads' channels; the host sums the two
partials per batch.

Per-core layouts (SBUF partition dim first):
  hT   [c, t]   LN(x) transposed via PE, bf16
  qT/kT [o, t]  o = head*64+d; head pair (2i,2i+1) shares a 128-partition tile
  v    [t, (h, 65)] bf16, col 64 = ones (PV emits softmax sums as row 64)
  scores sT [tk, tq] per 128x512 tile, K=64 head-pairs row-tiled concurrently;
  exp on ACT (scale=1/8 fused) over valid columns only (tq_loc >= r for the
  diagonal tile at offset r); causality via a single 128-wide multiplicative
  [i>j] mask on DVE; PV: lhsT=v_ext, rhs=p[:, r:] -> out2t [65, tq]
  normalization: sums -> DRAM roundtrip -> 64-partition broadcast -> DVE mul
  out_proj: lhsT = A.T [j, t] f32r, rhs = woT [j, o] f32r
"""
import math
import sys

sys.path.insert(0, "/opt/trn_rl_repo")
sys.path.insert(0, "/opt/trn_rl_repo/concourse")

import numpy as np
import ml_dtypes

import concourse.bass as bass
import concourse.bacc as bacc
import concourse.mybir as mybir
import concourse.tile as tile
from concourse.bass_utils import run_bass_kernel_spmd

T, C, NH, DH = 2048, 1024, 16, 64
HC = 8            # heads per core
NT = T // 128     # 16 t-tiles
KC = C // 128     # 8 contraction tiles
W = 512           # tq block width
NJ = T // W       # 4 q blocks
NP = HC // 2      # 4 head pairs
GS = 2            # kt tiles per scores/exp group
F32, F32R, BF16 = mybir.dt.float32, mybir.dt.float32r, mybir.dt.bfloat16
AF = mybir.ActivationFunctionType

_CACHE = {}


def _build(beta_nonzero):
    nc = bacc.Bacc("TRN2", target_bir_lowering=False, debug=False)
    dx = nc.dram_tensor("x", [T, C], F32, kind="ExternalInput")
    dwq = nc.dram_tensor("wq", [KC, 128, 512], BF16, kind="ExternalInput")
    dwk = nc.dram_tensor("wk", [KC, 128, 512], BF16, kind="ExternalInput")
    dwv = nc.dram_tensor("wv", [KC, 128, 512], BF16, kind="ExternalInput")
    dwo = nc.dram_tensor("wo", [NP, 128, 1024], F32R, kind="ExternalInput")
    dmask = nc.dram_tensor("masks", [4, 128, 512], BF16, kind="ExternalInput")
    did = nc.dram_tensor("ident", [128, 128], BF16, kind="ExternalInput")
    dbeta = nc.dram_tensor("betab", [1, C], F32, kind="ExternalInput")
    dout = nc.dram_tensor("out", [T, C], F32, kind="ExternalOutput")

    with tile.TileContext(nc) as tc:
        cst = tc.alloc_tile_pool(name="cst", bufs=1)
        ident = cst.tile([128, 128], BF16)
        mask_sb = cst.tile([128, 4, 512], BF16)
        wo_sb = cst.tile([128, NP, 1024], F32R)
        eps = cst.tile([128, 1], F32)
        nc.sync.dma_start(ident[:], did[:])
        nc.vector.memset(eps[:], 1e-5)
        att = tc.alloc_tile_pool(name="att", bufs=1)
        qT = att.tile([128, NP, T], BF16)
        kT = att.tile([128, NP, T], BF16)
        v_sb = att.tile([128, NT, HC, 65], BF16)
        nc.vector.memset(v_sb[:, :, :, 64:65], 1.0)

        # ---------------- Phase A: LN -> transpose -> QKV ----------------
        with tc.tile_pool(name="wqkv", bufs=1) as wp, \
             tc.tile_pool(name="xp", bufs=3) as xp, \
             tc.tile_pool(name="hp", bufs=3) as hp, \
             tc.tile_pool(name="hT", bufs=1) as hTp, \
             tc.tile_pool(name="st", bufs=4) as stp, \
             tc.tile_pool(name="tps", bufs=3, space="PSUM") as tps, \
             tc.tile_pool(name="qkps", bufs=4, space="PSUM") as qkps:
            wq_sb = wp.tile([128, KC, 512], BF16, tag="w")
            wk_sb = wp.tile([128, KC, 512], BF16, tag="w2")
            wv_sb = wp.tile([128, KC, 512], BF16, tag="w3")
            if beta_nonzero:
                beta_sb = wp.tile([128, C], F32, tag="beta")
                bap = dbeta[0:1, :]
                nc.gpsimd.dma_start(
                    out=beta_sb[:],
                    in_=bass.AP(tensor=bap.tensor, offset=bap.offset,
                                ap=[[0, 128], bap.ap[1]]))
            hT = hTp.tile([128, KC, T], BF16)
            for tb in range(NJ):
                for tt in range(4 * tb, 4 * tb + 4):
                    xt = xp.tile([128, C], F32)
                    nc.sync.dma_start(xt[:], dx[tt * 128:(tt + 1) * 128, :])
                    if tb == 0 and tt < 2:
                        for kc in range(4 * tt, 4 * tt + 4):
                            nc.sync.dma_start(wq_sb[:, kc, :], dwq[kc])
                            nc.sync.dma_start(wk_sb[:, kc, :], dwk[kc])
                            nc.sync.dma_start(wv_sb[:, kc, :], dwv[kc])
                    stats = stp.tile([128, 2, 6], F32, tag="stats")
                    xg = xt[:].rearrange("p (g d) -> p g d", g=2)
                    for g in range(2):
                        nc.vector.bn_stats(stats[:, g, :], xg[:, g, :])
                    mv = stp.tile([128, 2], F32, tag="mv")
                    nc.vector.bn_aggr(mv[:], stats[:])
                    sd = stp.tile([128, 1], F32, tag="sd")
                    nc.scalar.activation(sd[:], mv[:, 1:2], AF.Sqrt, bias=eps[:], scale=1.0)
                    nc.vector.reciprocal(sd[:], sd[:])
                    ht = hp.tile([128, C], BF16)
                    nc.vector.tensor_scalar(
                        out=ht[:], in0=xt[:], scalar1=mv[:, 0:1], scalar2=sd[:],
                        op0=mybir.AluOpType.subtract, op1=mybir.AluOpType.mult)
                    if beta_nonzero:
                        nc.vector.tensor_add(ht[:], ht[:], beta_sb[:])
                    tp = tps.tile([128, KC, 128], BF16)
                    for kc in range(KC):
                        nc.tensor.transpose(tp[:, kc, :], ht[:, kc * 128:(kc + 1) * 128], ident[:])
                    nc.vector.tensor_copy(hT[:, :, tt * 128:(tt + 1) * 128], tp[:])
                for ot in range(NP):
                    pq = qkps.tile([128, 512], F32, tag="ps")
                    for kc in range(KC):
                        nc.tensor.matmul(pq[:], wq_sb[:, kc, ot * 128:(ot + 1) * 128],
                                         hT[:, kc, tb * 512:(tb + 1) * 512],
                                         start=(kc == 0), stop=(kc == KC - 1))
                    nc.vector.tensor_copy(qT[:, ot, tb * 512:(tb + 1) * 512], pq[:])
                    pk = qkps.tile([128, 512], F32, tag="ps")
                    for kc in range(KC):
                        nc.tensor.matmul(pk[:], wk_sb[:, kc, ot * 128:(ot + 1) * 128],
                                         hT[:, kc, tb * 512:(tb + 1) * 512],
                                         start=(kc == 0), stop=(kc == KC - 1))
                    nc.vector.tensor_copy(kT[:, ot, tb * 512:(tb + 1) * 512], pk[:])
                for tt in range(4 * tb, 4 * tb + 4):
                    pv = qkps.tile([128, 512], F32, tag="ps")
                    for kc in range(KC):
                        nc.tensor.matmul(pv[:], hT[:, kc, tt * 128:(tt + 1) * 128],
                                         wv_sb[:, kc, :],
                                         start=(kc == 0), stop=(kc == KC - 1))
                    nc.vector.tensor_copy(
                        v_sb[:, tt, :, 0:64],
                        pv[:].rearrange("p (h d) -> p h d", h=HC))

        # ---------------- Phase B: attention + out_proj ----------------
        with tc.tile_pool(name="sps", bufs=3, space="PSUM") as sps, \
             tc.tile_pool(name="ops", bufs=2, space="PSUM") as ops, \
             tc.tile_pool(name="pp", bufs=6) as ppool, \
             tc.tile_pool(name="up", bufs=10) as upool, \
             tc.tile_pool(name="facp", bufs=8) as facp, \
             tc.tile_pool(name="atmp", bufs=4) as atmp, \
             tc.tile_pool(name="sums", bufs=1) as sums, \
             tc.tile_pool(name="atp", bufs=1) as atp, \
             tc.tile_pool(name="outp", bufs=4) as outp, \
             tc.tile_pool(name="drp", bufs=1, space="DRAM") as drp:
            for r in range(4):
                nc.sync.dma_start(mask_sb[:, r, :], dmask[r])
            for jp in range(NP):
                nc.sync.dma_start(wo_sb[:, jp, :], dwo[jp])
            s8 = sums.tile([8, NJ, 512], F32)
            recip8 = sums.tile([8, NJ, 512], F32)
            AT = atp.tile([128, NP, NJ, 512], F32R)
            drec = drp.tile([8, NJ, 512], F32)

            def emit_attention(J):
                nkt = 4 * J + 4
                u_tiles = []
                for hpair in range(NP):
                    hA, hB = 2 * hpair, 2 * hpair + 1
                    poA = ops.tile([65, 512], F32, tag="po")
                    poB = ops.tile([65, 512], F32, tag="po")
                    for g in range(nkt // GS):
                        kts = list(range(g * GS, (g + 1) * GS))
                        spA = sps.tile([128, GS, 512], F32, tag="sp")
                        spB = sps.tile([128, GS, 512], F32, tag="sp")
                        ptA = ppool.tile([128, GS, 512], BF16, tag="pt")
                        ptB = ppool.tile([128, GS, 512], BF16, tag="pt")
                        # column offset r: tq_loc < r is fully masked for
                        # diagonal tile kt (r = 128*(kt-4J)); skip those columns
                        offs = [max(0, (kt - 4 * J) * 128) for kt in kts]
                        for i, kt in enumerate(kts):
                            for sp, base in ((spA, 0), (spB, 64)):
                                nc.tensor.matmul(
                                    sp[:, i, :],
                                    kT[base:base + 64, hpair, kt * 128:(kt + 1) * 128],
                                    qT[base:base + 64, hpair, J * 512:(J + 1) * 512],
                                    start=True, stop=True,
                                    tile_position=(base, 0))
                        if offs == [0] * GS:
                            for sp, pt in ((spA, ptA), (spB, ptB)):
                                nc.scalar.activation(
                                    pt[:].rearrange("p g f -> p (g f)"),
                                    sp[:].rearrange("p g f -> p (g f)"),
                                    AF.Exp, scale=0.125)
                        else:
                            for i, kt in enumerate(kts):
                                for sp, pt in ((spA, ptA), (spB, ptB)):
                                    nc.scalar.activation(
                                        pt[:, i, offs[i]:512],
                                        sp[:, i, offs[i]:512],
                                        AF.Exp, scale=0.125)
                        for i, kt in enumerate(kts):
                            if kt - 4 * J >= 0:
                                r = offs[i]
                                for pt in (ptA, ptB):
                                    nc.vector.tensor_mul(pt[:, i, r:r + 128],
                                                         pt[:, i, r:r + 128],
                                                         mask_sb[:, 0, 0:128])
                        for i, kt in enumerate(kts):
                            r = offs[i]
                            for po, h, pt in ((poA, hA, ptA), (poB, hB, ptB)):
                                nc.tensor.matmul(
                                    po[:, r:512], v_sb[:, kt, h, :], pt[:, i, r:512],
                                    start=(kt == 0), stop=(kt == nkt - 1))
                    uA = upool.tile([65, 512], F32, tag="u")
                    uB = upool.tile([65, 512], F32, tag="u")
                    nc.vector.tensor_copy(uA[:], poA[:])
                    nc.vector.tensor_copy(uB[:], poB[:])
                    u_tiles.append((uA, uB))
                    nc.sync.dma_start(s8[hA:hA + 1, J, :], uA[64:65, :])
                    nc.sync.dma_start(s8[hB:hB + 1, J, :], uB[64:65, :])
                nc.vector.reciprocal(recip8[:, J, :], s8[:, J, :])
                nc.sync.dma_start(drec[:, J, :], recip8[:, J, :])
                for hpair in range(NP):
                    uA, uB = u_tiles[hpair]
                    for hh, h, u in ((0, 2 * hpair, uA), (1, 2 * hpair + 1, uB)):
                        fac = facp.tile([64, 512], F32)
                        row = drec[h:h + 1, J, :]
                        nc.sync.dma_start(
                            fac[:],
                            bass.AP(tensor=row.tensor, offset=row.offset,
                                    ap=[[0, 64], row.ap[-1]]))
                        if hh == 0:
                            nc.vector.tensor_mul(AT[0:64, hpair, J, :],
                                                 u[0:64, :], fac[:])
                        else:
                            at = atmp.tile([64, 512], F32R)
                            nc.vector.tensor_mul(at[:], u[0:64, :], fac[:])
                            nc.sync.dma_start(AT[64:128, hpair, J, :], at[:])

            def emit_out_proj(J):
                for tc4 in range(4):
                    for ob in range(2):
                        pp_ = sps.tile([128, 512], F32, tag="sp")
                        for hpair in range(NP):
                            nc.tensor.matmul(
                                pp_[:], AT[:, hpair, J, tc4 * 128:(tc4 + 1) * 128],
                                wo_sb[:, hpair, ob * 512:(ob + 1) * 512],
                                start=(hpair == 0), stop=(hpair == NP - 1))
                        ot_ = outp.tile([128, 512], F32)
                        nc.vector.tensor_copy(ot_[:], pp_[:])
                        t0 = J * 512 + tc4 * 128
                        nc.sync.dma_start(dout[t0:t0 + 128, ob * 512:(ob + 1) * 512],
                                          ot_[:])

            for J in range(NJ):
                emit_attention(J)
                if J > 0:
                    emit_out_proj(J - 1)
            emit_out_proj(NJ - 1)
        att.release()
        cst.release()
    nc.compile()
    return nc


def kernel(x, gamma, beta, w_qkv, w_out):
    x = np.asarray(x, dtype=np.float32)
    gamma = np.asarray(gamma, dtype=np.float32)
    beta = np.asarray(beta, dtype=np.float32)
    w_qkv = np.asarray(w_qkv, dtype=np.float32)
    w_out = np.asarray(w_out, dtype=np.float32)
    B = x.shape[0]
    beta_nonzero = bool(np.any(beta != 0.0))
    key = ("k", beta_nonzero)
    if key not in _CACHE:
        _CACHE[key] = _build(beta_nonzero)
    nc = _CACHE[key]

    i128, j128 = np.indices((128, 512))
    masks = np.stack([np.where(i128 + r > j128, 0.0, 1.0)
                      for r in (0, 128, 256, 384)]).astype(ml_dtypes.bfloat16)
    ident = np.eye(128, dtype=ml_dtypes.bfloat16)
    betab = beta.reshape(1, C)

    in_maps = []
    for core in range(8):
        b, g = core // 2, core % 2
        sl = slice(g * 512, (g + 1) * 512)
        wq = (w_qkv[0 * C:1 * C][sl] * gamma[None, :]).T.copy()      # [1024, 512]
        wk = (w_qkv[1 * C:2 * C][sl] * gamma[None, :]).T.copy()
        wv = (w_qkv[2 * C:3 * C][sl] * gamma[None, :]).T.copy()
        wo = w_out[:, sl].T.copy()                                    # [512, 1024]
        in_maps.append({
            "x": np.ascontiguousarray(x[b]),
            "wq": wq.reshape(KC, 128, 512).astype(ml_dtypes.bfloat16),
            "wk": wk.reshape(KC, 128, 512).astype(ml_dtypes.bfloat16),
            "wv": wv.reshape(KC, 128, 512).astype(ml_dtypes.bfloat16),
            "wo": np.ascontiguousarray(wo.reshape(NP, 128, 1024)),
            "masks": masks,
            "ident": ident,
            "betab": betab,
        })
    res = run_bass_kernel_spmd(nc, in_maps, core_ids=list(range(8)))
    out = np.empty((B, T, C), dtype=np.float32)
    for b in range(B):
        out[b] = res.results[2 * b]["out"] + res.results[2 * b + 1]["out"]
    return out



# revision 16
# speedup vs baseline: 1.1124x; 1.1124x over previous
"""CausalSelfAttention TRN2 kernel: LN + QKV + causal attention + out_proj.

Sharding: 8 cores = 4 batches x 2 head-groups (8 heads each). Each core
computes its batch's LayerNorm, QKV for its heads, causal softmax attention,
and a partial out-projection over its heads' channels; the host sums the two
partials per batch.

Schedule: single woven instruction stream. Per-tt LayerNorm/transpose/QKV
chains act as PE filler units interleaved into the attention J-blocks so the
PE never starves while ACT runs exp. Normalization avoids all HWDGE traffic:
sums ride the PV matmul (ones column in v), the B-half PSUM is moved to
partitions 63..127 by a gpsimd SWDGE DMA, reciprocal factors are broadcast
across partitions with gpsimd partition_broadcast, and out_proj DMAs its
PSUM tiles straight to DRAM.

Per-core layouts (SBUF partition dim first):
  hT   [c, t]   LN(x) transposed via PE, bf16, per-tt tiles
  qT/kT [o, t]  o = head*64+d; head pair (2i,2i+1) shares a 128-partition tile
  v    [t, (h, 65)] bf16, col 64 = ones (PV emits softmax sums as row 64)
  scores sp [tk, 2, tq] per kt tile (both pair halves share one PSUM tile so
  one ACT exp covers them); causal diagonal tiles slice rhs to [offs:512];
  causality inside the first 128 cols via a multiplicative [i>j] mask on DVE.
  out_proj: lhsT = AT [j, t] bf16, rhs = woT [j, o] bf16, PSUM -> DRAM DMA.
"""
import math
import sys
from collections import deque

sys.path.insert(0, "/opt/trn_rl_repo")
sys.path.insert(0, "/opt/trn_rl_repo/concourse")

import numpy as np
import ml_dtypes

import concourse.bass as bass
import concourse.bacc as bacc
import concourse.mybir as mybir
import concourse.tile as tile
from concourse.bass_utils import run_bass_kernel_spmd

T, C, NH, DH = 2048, 1024, 16, 64
HC = 8            # heads per core
NT = T // 128     # 16 t-tiles
KC = C // 128     # 8 contraction tiles
W = 512           # tq block width
NJ = T // W       # 4 q blocks
NP = HC // 2      # 4 head pairs
F32, BF16 = mybir.dt.float32, mybir.dt.bfloat16
AF = mybir.ActivationFunctionType
SUB, MUL = mybir.AluOpType.subtract, mybir.AluOpType.mult

_CACHE = {}
DEBUG = False


def _build(beta_nonzero):
    nc = bacc.Bacc("TRN2", target_bir_lowering=False, debug=False)
    dx = nc.dram_tensor("x", [T, C], F32, kind="ExternalInput")
    dwq = nc.dram_tensor("wq", [KC, 128, 512], BF16, kind="ExternalInput")
    dwk = nc.dram_tensor("wk", [KC, 128, 512], BF16, kind="ExternalInput")
    dwv = nc.dram_tensor("wv", [KC, 128, 512], BF16, kind="ExternalInput")
    dwo = nc.dram_tensor("wo", [NP, 128, 1024], BF16, kind="ExternalInput")
    dmask = nc.dram_tensor("masks", [128, 128], BF16, kind="ExternalInput")
    did = nc.dram_tensor("ident", [128, 128], BF16, kind="ExternalInput")
    dbeta = nc.dram_tensor("betab", [1, C], F32, kind="ExternalInput")
    dout = nc.dram_tensor("out", [T, C], F32, kind="ExternalOutput")
    if DEBUG:
        dbg_q = nc.dram_tensor("dbg_q", [128, NP, T], BF16, kind="ExternalOutput")
        dbg_k = nc.dram_tensor("dbg_k", [128, NP, T], BF16, kind="ExternalOutput")
        dbg_v = nc.dram_tensor("dbg_v", [128, NT, HC, 65], BF16, kind="ExternalOutput")
        dbg_at = nc.dram_tensor("dbg_at", [128, NP, NJ, 512], BF16, kind="ExternalOutput")
        dbg_ua = nc.dram_tensor("dbg_ua", [NJ, NP, 65, 512], F32, kind="ExternalOutput")
        dbg_uh = nc.dram_tensor("dbg_uh", [NJ, NP, 128, 512], F32, kind="ExternalOutput")
        dbg_fac = nc.dram_tensor("dbg_fac", [NJ, NP, 128, 512], F32, kind="ExternalOutput")

    with tile.TileContext(nc) as tc:
        cst = tc.alloc_tile_pool(name="cst", bufs=1)
        ident = cst.tile([128, 128], BF16)
        mask1 = cst.tile([128, 128], BF16)
        eps = cst.tile([128, 1], F32)
        wq_sb = cst.tile([128, KC, 512], BF16)
        wk_sb = cst.tile([128, KC, 512], BF16)
        wv_sb = cst.tile([128, KC, 512], BF16)
        wo_sb = cst.tile([128, NP, 1024], BF16)
        qT = cst.tile([128, NP, T], BF16)
        kT = cst.tile([128, NP, T], BF16)
        v_sb = cst.tile([128, NT, HC, 65], BF16)
        AT = cst.tile([128, NP, NJ, 512], BF16)
        if beta_nonzero:
            beta_sb = cst.tile([128, C], F32)

        nc.vector.memset(eps[:], 1e-5)
        nc.vector.memset(v_sb[:, :, :, 64:65], 1.0)

        xp = tc.alloc_tile_pool(name="xp", bufs=6)
        hp = tc.alloc_tile_pool(name="hp", bufs=3)
        htp = tc.alloc_tile_pool(name="htp", bufs=8)
        stp = tc.alloc_tile_pool(name="stp", bufs=4)
        ptp = tc.alloc_tile_pool(name="ptp", bufs=6)
        uap = tc.alloc_tile_pool(name="uap", bufs=6)
        uhp = tc.alloc_tile_pool(name="uhp", bufs=4)
        facp = tc.alloc_tile_pool(name="facp", bufs=4)
        outp = tc.alloc_tile_pool(name="outp", bufs=4)
        pap = tc.alloc_tile_pool(name="pap", bufs=2, space="PSUM")
        spp = tc.alloc_tile_pool(name="spp", bufs=2, space="PSUM")
        pop = tc.alloc_tile_pool(name="pop", bufs=2, space="PSUM")
        drp = tc.alloc_tile_pool(name="drp", bufs=1, space="DRAM")
        drec = drp.tile([NJ, NP, 2, 512], F32)

        # ---- DMA issue: sync/HWDGE queue (mask, ident, weights, x8-15, wo)
        nc.sync.dma_start(mask1[:], dmask[:])
        nc.sync.dma_start(ident[:], did[:])
        xts = []
        for tt in range(NT):
            xt = xp.tile([128, C], F32, tag="x", name=f"xt{tt}")
            xts.append(xt)
        # ---- Pool/SWDGE queue: x0-7 (emitted first: slot writers must appear
        # in allocation order)
        for tt in range(0, 8):
            nc.gpsimd.dma_start(xts[tt][:], dx[tt * 128:(tt + 1) * 128, :])
        for kc in range(KC):
            nc.sync.dma_start(wq_sb[:, kc, :], dwq[kc])
        for kc in range(KC):
            nc.sync.dma_start(wk_sb[:, kc, :], dwk[kc])
        for kc in range(KC):
            nc.sync.dma_start(wv_sb[:, kc, :], dwv[kc])
        for tt in range(8, NT):
            nc.sync.dma_start(xts[tt][:], dx[tt * 128:(tt + 1) * 128, :])
        for jp in range(NP):
            nc.sync.dma_start(wo_sb[:, jp, :], dwo[jp])
        if beta_nonzero:
            bap = dbeta[0:1, :]
            nc.sync.dma_start(
                beta_sb[:],
                bass.AP(tensor=bap.tensor, offset=bap.offset,
                        ap=[[0, 128], bap.ap[1]]))

        hTs = [None] * NT

        def emit_ln(tt):
            xt = xts[tt]
            stats = stp.tile([128, 2, 6], F32, tag="st")
            xg = xt[:].rearrange("p (g d) -> p g d", g=2)
            for g in range(2):
                nc.vector.bn_stats(stats[:, g, :], xg[:, g, :])
            mv = stp.tile([128, 2], F32, tag="mv")
            nc.vector.bn_aggr(mv[:], stats[:])
            sd = stp.tile([128, 1], F32, tag="sd")
            nc.scalar.activation(sd[:], mv[:, 1:2], AF.Sqrt, bias=eps[:], scale=1.0)
            nc.vector.reciprocal(sd[:], sd[:])
            ht = hp.tile([128, C], BF16, tag="h")
            nc.vector.tensor_scalar(
                out=ht[:], in0=xt[:], scalar1=mv[:, 0:1], scalar2=sd[:],
                op0=SUB, op1=MUL)
            if beta_nonzero:
                nc.vector.tensor_add(ht[:], ht[:], beta_sb[:])
            tp = pap.tile([128, KC, 128], BF16, tag="pa", name="tp")
            for kc in range(KC):
                nc.tensor.transpose(tp[:, kc, :], ht[:, kc * 128:(kc + 1) * 128], ident[:])
            hT = htp.tile([128, KC, 128], BF16, tag="hT", name=f"hT{tt}")
            nc.vector.tensor_copy(hT[:], tp[:])
            hTs[tt] = hT

        def emit_q(tt):
            q4 = pap.tile([128, NP, 128], F32, tag="pa", name="q4")
            for ot in range(NP):
                for kc in range(KC):
                    nc.tensor.matmul(q4[:, ot, :],
                                     wq_sb[:, kc, ot * 128:(ot + 1) * 128],
                                     hTs[tt][:, kc, :],
                                     start=(kc == 0), stop=(kc == KC - 1))
            nc.vector.tensor_copy(qT[:, :, tt * 128:(tt + 1) * 128], q4[:])

        def emit_k(tt):
            k4 = pap.tile([128, NP, 128], F32, tag="pa", name="k4")
            for ot in range(NP):
                for kc in range(KC):
                    nc.tensor.matmul(k4[:, ot, :],
                                     wk_sb[:, kc, ot * 128:(ot + 1) * 128],
                                     hTs[tt][:, kc, :],
                                     start=(kc == 0), stop=(kc == KC - 1))
            nc.vector.tensor_copy(kT[:, :, tt * 128:(tt + 1) * 128], k4[:])

        def emit_v(tt):
            v1 = pap.tile([128, 512], F32, tag="pa", name="v1")
            for kc in range(KC):
                nc.tensor.matmul(v1[:], hTs[tt][:, kc, :], wv_sb[:, kc, :],
                                 start=(kc == 0), stop=(kc == KC - 1))
            nc.vector.tensor_copy(
                v_sb[:, tt, :, 0:64],
                v1[:].rearrange("p (h d) -> p h d", h=HC))

        def emit_out(J, tc4, ob):
            pp = pap.tile([128, 512], F32, tag="pa", name="pp")
            for hp_ in range(NP):
                nc.tensor.matmul(pp[:],
                                 AT[:, hp_, J, tc4 * 128:(tc4 + 1) * 128],
                                 wo_sb[:, hp_, ob * 512:(ob + 1) * 512],
                                 start=(hp_ == 0), stop=(hp_ == NP - 1))
            ot_ = outp.tile([128, 512], F32, tag="ot", name="ot")
            nc.vector.tensor_copy(ot_[:], pp[:])
            t0 = J * 512 + tc4 * 128
            nc.sync.dma_start(dout[t0:t0 + 128, ob * 512:(ob + 1) * 512], ot_[:])

        # ---------------- attention ----------------
        def emit_att_kt(J, hpair, kt, nkt, poA, poB):
            offs = max(0, (kt - 4 * J) * 128)
            for half, base in ((0, 0), (1, 64)):
                sp = spAB[0]
                nc.tensor.matmul(
                    sp[:, half, offs:512],
                    kT[base:base + 64, hpair, kt * 128:(kt + 1) * 128],
                    qT[base:base + 64, hpair, J * 512 + offs:(J + 1) * 512],
                    start=True, stop=True,
                    tile_position=(base, 0))
            sp = spAB[0]
            pt = ptp.tile([128, 2, 512], BF16, tag="pt", name="pt")
            if offs == 0:
                nc.scalar.activation(
                    pt[:].rearrange("p g f -> p (g f)"),
                    sp[:].rearrange("p g f -> p (g f)"),
                    AF.Exp, scale=0.125)
            else:
                nc.scalar.activation(pt[:, :, offs:512], sp[:, :, offs:512],
                                     AF.Exp, scale=0.125)
            if kt - 4 * J >= 0:
                r = offs
                for half in range(2):
                    nc.vector.tensor_mul(pt[:, half, r:r + 128],
                                         pt[:, half, r:r + 128],
                                         mask1[:, 0:128])
            for half, po in ((0, poA), (1, poB)):
                h = 2 * hpair + half
                nc.tensor.matmul(po[:, offs:512], v_sb[:, kt, h, :],
                                 pt[:, half, offs:512],
                                 start=(kt == 0), stop=(kt == nkt - 1))

        spAB = [None]

        def emit_scores_tile():
            spAB[0] = spp.tile([128, 2, 512], F32, tag="sp", name="sp")

        def emit_norm1(J, hpair, poA, poB):
            # u copies (DVE, PSUM->SBUF), B-half U rows -> partitions 64..127
            # (gpsimd SWDGE SBUF->SBUF), sums-row broadcasts (gpsimd) into fac.
            uA = uap.tile([65, 512], F32, tag="ua", name="uA")
            nc.vector.tensor_copy(uA[:], poA[:])
            uB = uap.tile([65, 512], F32, tag="ua", name="uB")
            nc.vector.tensor_copy(uB[:], poB[:])
            uhi = uhp.tile([128, 512], F32, tag="uh", name="uhi")
            nc.gpsimd.dma_start(uhi[64:128, :], uB[0:64, :])
            # sums rows -> DRAM, then one broadcast load into fac (SBUF src
            # partition dim cannot have stride 0; DRAM src can repeat).
            nc.gpsimd.dma_start(drec[J, hpair, 0:1, :], uA[64:65, :])
            nc.gpsimd.dma_start(drec[J, hpair, 1:2, :], uB[64:65, :])
            fac = facp.tile([128, 512], F32, tag="fac", name="fac")
            rows = drec[J, hpair]
            nc.gpsimd.dma_start(
                fac[:],
                bass.AP(tensor=rows.tensor, offset=rows.offset,
                        ap=[[512, 2], [0, 64], [1, 512]]))
            return uA, uhi, fac

        def emit_norm2(J, hpair, uA, uhi, fac):
            nc.vector.reciprocal(fac[:], fac[:])
            if DEBUG:
                nc.sync.dma_start(dbg_ua[J, hpair], uA[:])
                nc.sync.dma_start(dbg_uh[J, hpair], uhi[:])
                nc.sync.dma_start(dbg_fac[J, hpair], fac[:])
            nc.vector.tensor_mul(AT[0:64, hpair, J, :], uA[0:64, :], fac[0:64, :])
            nc.vector.tensor_mul(AT[64:128, hpair, J, :], uhi[64:128, :],
                                 fac[64:128, :])

        # ---------------- woven emission ----------------
        fq = deque()
        for tt in range(NT):
            fq.append(lambda tt=tt: emit_ln(tt))
            fq.append(lambda tt=tt: emit_q(tt))
            if tt - 1 >= 0:
                fq.append(lambda tt=tt: emit_k(tt - 1))
            if tt - 3 >= 0:
                fq.append(lambda tt=tt: emit_v(tt - 3))
        fq.append(lambda: emit_k(NT - 1))
        for tt in range(NT - 3, NT):
            fq.append(lambda tt=tt: emit_v(tt))

        emitted = [0]

        def filler(n=1):
            for _ in range(n):
                if fq:
                    fq.popleft()()
                    emitted[0] += 1

        # qkv-unit watermark required before att(J): v(4J+3) must be emitted.
        NEED = [24, 40, 56, 64]

        pending2 = None          # deferred norm2 args
        pending_out = None       # J whose out chains go to fq next
        for J in range(NJ):
            nkt = 4 * J + 4
            filler(max(0, NEED[J] - emitted[0]))
            for hpair in range(NP):
                filler(1)
                poA = pop.tile([65, 512], F32, tag="po", name="poA")
                poB = pop.tile([65, 512], F32, tag="po", name="poB")
                for kt in range(nkt):
                    emit_scores_tile()
                    emit_att_kt(J, hpair, kt, nkt, poA, poB)
                    if kt == 1 and pending2 is not None:
                        emit_norm2(*pending2)
                        pending2 = None
                        if pending_out is not None:
                            Jo = pending_out
                            for tc4 in range(4):
                                for ob in range(2):
                                    fq.append(lambda Jo=Jo, tc4=tc4, ob=ob:
                                              emit_out(Jo, tc4, ob))
                            pending_out = None
                    if kt % 3 == 2:
                        filler(1)
                n1 = emit_norm1(J, hpair, poA, poB)
                if pending2 is not None:
                    emit_norm2(*pending2)
                pending2 = (J, hpair) + n1
            pending_out = J
        emit_norm2(*pending2)
        for tc4 in range(4):
            for ob in range(2):
                fq.append(lambda tc4=tc4, ob=ob: emit_out(NJ - 1, tc4, ob))
        filler(len(fq))
        if DEBUG:
            nc.sync.dma_start(dbg_q[:], qT[:])
            nc.sync.dma_start(dbg_k[:], kT[:])
            nc.sync.dma_start(dbg_v[:], v_sb[:])
            nc.sync.dma_start(dbg_at[:], AT[:])

        for p in (drp, pop, spp, pap, outp, facp, uhp, uap, ptp, stp, htp, hp, xp, cst):
            p.release()
    nc.compile()
    return nc


def kernel(x, gamma, beta, w_qkv, w_out):
    x = np.asarray(x, dtype=np.float32)
    gamma = np.asarray(gamma, dtype=np.float32)
    beta = np.asarray(beta, dtype=np.float32)
    w_qkv = np.asarray(w_qkv, dtype=np.float32)
    w_out = np.asarray(w_out, dtype=np.float32)
    B = x.shape[0]
    beta_nonzero = bool(np.any(beta != 0.0))
    key = ("k", beta_nonzero)
    if key not in _CACHE:
        _CACHE[key] = _build(beta_nonzero)
    nc = _CACHE[key]

    i128, j128 = np.indices((128, 128))
    mask = np.where(i128 > j128, 0.0, 1.0).astype(ml_dtypes.bfloat16)
    ident = np.eye(128, dtype=ml_dtypes.bfloat16)
    betab = beta.reshape(1, C)

    in_maps = []
    for core in range(8):
        b, g = core // 2, core % 2
        sl = slice(g * 512, (g + 1) * 512)
        wq = (w_qkv[0 * C:1 * C][sl] * gamma[None, :]).T.copy()      # [1024, 512]
        wk = (w_qkv[1 * C:2 * C][sl] * gamma[None, :]).T.copy()
        wv = (w_qkv[2 * C:3 * C][sl] * gamma[None, :]).T.copy()
        wo = w_out[:, sl].T.copy()                                    # [512, 1024]
        in_maps.append({
            "x": np.ascontiguousarray(x[b]),
            "wq": wq.reshape(KC, 128, 512).astype(ml_dtypes.bfloat16),
            "wk": wk.reshape(KC, 128, 512).astype(ml_dtypes.bfloat16),
            "wv": wv.reshape(KC, 128, 512).astype(ml_dtypes.bfloat16),
            "wo": wo.reshape(NP, 128, 1024).astype(ml_dtypes.bfloat16),
            "masks": mask,
            "ident": ident,
            "betab": betab,
        })
    res = run_bass_kernel_spmd(nc, in_maps, core_ids=list(range(8)))
    out = np.empty((B, T, C), dtype=np.float32)
    for b in range(B):
        out[b] = res.results[2 * b]["out"] + res.results[2 * b + 1]["out"]
    return out


# revision 26
# speedup vs baseline: 1.1286x; 1.0146x over previous
"""CausalSelfAttention TRN2 kernel: LN + QKV + causal attention + out_proj.

Sharding: 8 cores = 4 batches x 2 head-groups (8 heads each). Each core
computes its batch's LayerNorm, QKV for its heads, causal softmax attention,
and a partial out-projection over its heads' channels; the host sums the two
partials per batch.

Schedule: single woven instruction stream. Per-tt LayerNorm/transpose/QKV
chains act as PE filler units interleaved into the attention J-blocks so the
PE never starves while ACT runs exp. Normalization avoids all HWDGE traffic:
sums ride the PV matmul (ones column in v), the B-half PSUM is moved to
partitions 63..127 by a gpsimd SWDGE DMA, reciprocal factors are broadcast
across partitions with gpsimd partition_broadcast, and out_proj DMAs its
PSUM tiles straight to DRAM.

Per-core layouts (SBUF partition dim first):
  hT   [c, t]   LN(x) transposed via PE, bf16, per-tt tiles
  qT/kT [o, t]  o = head*64+d; head pair (2i,2i+1) shares a 128-partition tile
  v    [t, (h, 65)] bf16, col 64 = ones (PV emits softmax sums as row 64)
  scores sp [tk, 2, tq] per kt tile (both pair halves share one PSUM tile so
  one ACT exp covers them); causal diagonal tiles slice rhs to [offs:512];
  causality inside the first 128 cols via a multiplicative [i>j] mask on DVE.
  out_proj: lhsT = AT [j, t] bf16, rhs = woT [j, o] bf16, PSUM -> DRAM DMA.
"""
import math
import sys
from collections import deque

sys.path.insert(0, "/opt/trn_rl_repo")
sys.path.insert(0, "/opt/trn_rl_repo/concourse")

import numpy as np
import ml_dtypes

import concourse.bass as bass
import concourse.bacc as bacc
import concourse.mybir as mybir
import concourse.tile as tile
from concourse.bass_utils import run_bass_kernel_spmd

T, C, NH, DH = 2048, 1024, 16, 64
HC = 8            # heads per core
NT = T // 128     # 16 t-tiles
KC = C // 128     # 8 contraction tiles
W = 512           # tq block width
NJ = T // W       # 4 q blocks
NP = HC // 2      # 4 head pairs
F32, BF16 = mybir.dt.float32, mybir.dt.bfloat16
AF = mybir.ActivationFunctionType
SUB, MUL = mybir.AluOpType.subtract, mybir.AluOpType.mult

_CACHE = {}
DEBUG = False


def _build(beta_nonzero):
    nc = bacc.Bacc("TRN2", target_bir_lowering=False, debug=False)
    dx = nc.dram_tensor("x", [T, C], F32, kind="ExternalInput")
    dwq = nc.dram_tensor("wq", [KC, 128, 512], BF16, kind="ExternalInput")
    dwk = nc.dram_tensor("wk", [KC, 128, 512], BF16, kind="ExternalInput")
    dwv = nc.dram_tensor("wv", [KC, 128, 512], BF16, kind="ExternalInput")
    dwo = nc.dram_tensor("wo", [NP, 128, 1024], BF16, kind="ExternalInput")
    dmask = nc.dram_tensor("masks", [128, 128], BF16, kind="ExternalInput")
    did = nc.dram_tensor("ident", [128, 128], BF16, kind="ExternalInput")
    dbeta = nc.dram_tensor("betab", [1, C], F32, kind="ExternalInput")
    dout = nc.dram_tensor("out", [T, C], F32, kind="ExternalOutput")
    if DEBUG:
        dbg_q = nc.dram_tensor("dbg_q", [128, NP, T], BF16, kind="ExternalOutput")
        dbg_k = nc.dram_tensor("dbg_k", [128, NP, T], BF16, kind="ExternalOutput")
        dbg_v = nc.dram_tensor("dbg_v", [128, NT, HC, 65], BF16, kind="ExternalOutput")
        dbg_at = nc.dram_tensor("dbg_at", [128, NP, NJ, 512], BF16, kind="ExternalOutput")
        dbg_ua = nc.dram_tensor("dbg_ua", [NJ, NP, 65, 512], F32, kind="ExternalOutput")
        dbg_uh = nc.dram_tensor("dbg_uh", [NJ, NP, 128, 512], F32, kind="ExternalOutput")
        dbg_fac = nc.dram_tensor("dbg_fac", [NJ, NP, 128, 512], F32, kind="ExternalOutput")

    with tile.TileContext(nc) as tc:
        cst = tc.alloc_tile_pool(name="cst", bufs=1)
        ident = cst.tile([128, 128], BF16)
        mask1 = cst.tile([128, 128], BF16)
        eps = cst.tile([128, 1], F32)
        wq_sb = cst.tile([128, KC, 512], BF16)
        wk_sb = cst.tile([128, KC, 512], BF16)
        wv_sb = cst.tile([128, KC, 512], BF16)
        wo_sb = cst.tile([128, NP, 1024], BF16)
        qT = cst.tile([128, NP, T], BF16)
        kT = cst.tile([128, NP, T], BF16)
        v_sb = cst.tile([128, NT, HC, 65], BF16)
        AT = cst.tile([128, NP, NJ, 512], BF16)
        if beta_nonzero:
            beta_sb = cst.tile([128, C], F32)

        nc.vector.memset(eps[:], 1e-5)
        nc.vector.memset(v_sb[:, :, :, 64:65], 1.0)

        xp = tc.alloc_tile_pool(name="xp", bufs=6)
        hp = tc.alloc_tile_pool(name="hp", bufs=3)
        htp = tc.alloc_tile_pool(name="htp", bufs=8)
        stp = tc.alloc_tile_pool(name="stp", bufs=4)
        ptp = tc.alloc_tile_pool(name="ptp", bufs=6)
        uap = tc.alloc_tile_pool(name="uap", bufs=6)
        uhp = tc.alloc_tile_pool(name="uhp", bufs=4)
        facp = tc.alloc_tile_pool(name="facp", bufs=4)
        outp = tc.alloc_tile_pool(name="outp", bufs=4)
        pap = tc.alloc_tile_pool(name="pap", bufs=2, space="PSUM")
        spp = tc.alloc_tile_pool(name="spp", bufs=2, space="PSUM")
        pop = tc.alloc_tile_pool(name="pop", bufs=2, space="PSUM")
        drp = tc.alloc_tile_pool(name="drp", bufs=1, space="DRAM")
        drec = drp.tile([NJ, NP, 2, 512], F32)

        # ---- DMA issue: sync/HWDGE queue (x0-1, weights, x8-15, wo)
        xts = []
        for tt in range(NT):
            xt = xp.tile([128, C], F32, tag="x", name=f"xt{tt}")
            xts.append(xt)
        # ident first (first transposes need it ~6us), then x0/x1 on sync
        # (earliest arrival), x2-7 on Pool/SWDGE. Writers must appear in
        # allocation order per slot, so keep tt order.
        nc.sync.dma_start(ident[:], did[:])
        nc.sync.dma_start(xts[0][:], dx[0:128, :])
        nc.sync.dma_start(xts[1][:], dx[128:256, :])
        for tt in range(2, 8):
            nc.gpsimd.dma_start(xts[tt][:], dx[tt * 128:(tt + 1) * 128, :])
        for kc in range(KC):
            nc.sync.dma_start(wq_sb[:, kc, :], dwq[kc])
        for kc in range(KC):
            nc.sync.dma_start(wk_sb[:, kc, :], dwk[kc])
        nc.sync.dma_start(mask1[:], dmask[:])
        for kc in range(KC):
            nc.sync.dma_start(wv_sb[:, kc, :], dwv[kc])
        for tt in range(8, NT):
            nc.sync.dma_start(xts[tt][:], dx[tt * 128:(tt + 1) * 128, :])
        for jp in range(NP):
            nc.sync.dma_start(wo_sb[:, jp, :], dwo[jp])
        if beta_nonzero:
            bap = dbeta[0:1, :]
            nc.sync.dma_start(
                beta_sb[:],
                bass.AP(tensor=bap.tensor, offset=bap.offset,
                        ap=[[0, 128], bap.ap[1]]))

        hTs = [None] * NT

        def emit_ln(tt):
            xt = xts[tt]
            stats = stp.tile([128, 2, 6], F32, tag="st")
            xg = xt[:].rearrange("p (g d) -> p g d", g=2)
            for g in range(2):
                nc.vector.bn_stats(stats[:, g, :], xg[:, g, :])
            mv = stp.tile([128, 2], F32, tag="mv")
            nc.vector.bn_aggr(mv[:], stats[:])
            sd = stp.tile([128, 1], F32, tag="sd")
            nc.scalar.activation(sd[:], mv[:, 1:2], AF.Sqrt, bias=eps[:], scale=1.0)
            nc.vector.reciprocal(sd[:], sd[:])
            ht = hp.tile([128, C], BF16, tag="h")
            nc.vector.tensor_scalar(
                out=ht[:], in0=xt[:], scalar1=mv[:, 0:1], scalar2=sd[:],
                op0=SUB, op1=MUL)
            if beta_nonzero:
                nc.vector.tensor_add(ht[:], ht[:], beta_sb[:])
            tp = pap.tile([128, KC, 128], BF16, tag="pa", name="tp")
            for kc in range(KC):
                nc.tensor.transpose(tp[:, kc, :], ht[:, kc * 128:(kc + 1) * 128], ident[:])
            hT = htp.tile([128, KC, 128], BF16, tag="hT", name=f"hT{tt}")
            nc.vector.tensor_copy(hT[:], tp[:])
            hTs[tt] = hT

        def emit_q(tt):
            q4 = pap.tile([128, NP, 128], F32, tag="pa", name="q4")
            for ot in range(NP):
                for kc in range(KC):
                    nc.tensor.matmul(q4[:, ot, :],
                                     wq_sb[:, kc, ot * 128:(ot + 1) * 128],
                                     hTs[tt][:, kc, :],
                                     start=(kc == 0), stop=(kc == KC - 1))
            nc.vector.tensor_copy(qT[:, :, tt * 128:(tt + 1) * 128], q4[:])

        def emit_k(tt):
            k4 = pap.tile([128, NP, 128], F32, tag="pa", name="k4")
            for ot in range(NP):
                for kc in range(KC):
                    nc.tensor.matmul(k4[:, ot, :],
                                     wk_sb[:, kc, ot * 128:(ot + 1) * 128],
                                     hTs[tt][:, kc, :],
                                     start=(kc == 0), stop=(kc == KC - 1))
            nc.vector.tensor_copy(kT[:, :, tt * 128:(tt + 1) * 128], k4[:])

        def emit_v(tt):
            v1 = pap.tile([128, 512], F32, tag="pa", name="v1")
            for kc in range(KC):
                nc.tensor.matmul(v1[:], hTs[tt][:, kc, :], wv_sb[:, kc, :],
                                 start=(kc == 0), stop=(kc == KC - 1))
            nc.vector.tensor_copy(
                v_sb[:, tt, :, 0:64],
                v1[:].rearrange("p (h d) -> p h d", h=HC))

        def emit_out(J, tc4, ob):
            pp = pap.tile([128, 512], F32, tag="pa", name="pp")
            for hp_ in range(NP):
                nc.tensor.matmul(pp[:],
                                 AT[:, hp_, J, tc4 * 128:(tc4 + 1) * 128],
                                 wo_sb[:, hp_, ob * 512:(ob + 1) * 512],
                                 start=(hp_ == 0), stop=(hp_ == NP - 1))
            ot_ = outp.tile([128, 512], F32, tag="ot", name="ot")
            nc.vector.tensor_copy(ot_[:], pp[:])
            t0 = J * 512 + tc4 * 128
            nc.sync.dma_start(dout[t0:t0 + 128, ob * 512:(ob + 1) * 512], ot_[:])

        # ---------------- attention ----------------
        def emit_att_kt(J, hpair, kt, nkt, poA, poB):
            offs = max(0, (kt - 4 * J) * 128)
            for half, base in ((0, 0), (1, 64)):
                sp = spAB[0]
                nc.tensor.matmul(
                    sp[:, half, offs:512],
                    kT[base:base + 64, hpair, kt * 128:(kt + 1) * 128],
                    qT[base:base + 64, hpair, J * 512 + offs:(J + 1) * 512],
                    start=True, stop=True,
                    tile_position=(base, 0))
            sp = spAB[0]
            pt = ptp.tile([128, 2, 512], BF16, tag="pt", name="pt")
            if offs == 0:
                nc.scalar.activation(
                    pt[:].rearrange("p g f -> p (g f)"),
                    sp[:].rearrange("p g f -> p (g f)"),
                    AF.Exp, scale=0.125)
            else:
                nc.scalar.activation(pt[:, :, offs:512], sp[:, :, offs:512],
                                     AF.Exp, scale=0.125)
            if kt - 4 * J >= 0:
                r = offs
                for half in range(2):
                    nc.vector.tensor_mul(pt[:, half, r:r + 128],
                                         pt[:, half, r:r + 128],
                                         mask1[:, 0:128])
            for half, po in ((0, poA), (1, poB)):
                h = 2 * hpair + half
                nc.tensor.matmul(po[:, offs:512], v_sb[:, kt, h, :],
                                 pt[:, half, offs:512],
                                 start=(kt == 0), stop=(kt == nkt - 1))

        spAB = [None]

        def emit_scores_tile():
            spAB[0] = spp.tile([128, 2, 512], F32, tag="sp", name="sp")

        def emit_norm1(J, hpair, poA, poB):
            # u copies (DVE, PSUM->SBUF), B-half U rows -> partitions 64..127
            # (gpsimd SWDGE SBUF->SBUF), sums-row broadcasts (gpsimd) into fac.
            uA = uap.tile([65, 512], F32, tag="ua", name="uA")
            nc.vector.tensor_copy(uA[:], poA[:])
            uB = uap.tile([65, 512], F32, tag="ua", name="uB")
            nc.vector.tensor_copy(uB[:], poB[:])
            uhi = uhp.tile([128, 512], F32, tag="uh", name="uhi")
            # Tail hpair: spread DMAs over idle low-latency queues instead of
            # serializing ~4us of desc-gen on the Pool engine.
            last = (J == NJ - 1 and hpair == NP - 1)
            qs = (nc.sync, nc.scalar, nc.scalar, nc.sync) if last else (
                nc.gpsimd, nc.gpsimd, nc.gpsimd, nc.gpsimd)
            qs[0].dma_start(uhi[64:128, :], uB[0:64, :])
            # sums rows -> DRAM, then one broadcast load into fac (SBUF src
            # partition dim cannot have stride 0; DRAM src can repeat).
            qs[1].dma_start(drec[J, hpair, 0:1, :], uA[64:65, :])
            qs[2].dma_start(drec[J, hpair, 1:2, :], uB[64:65, :])
            fac = facp.tile([128, 512], F32, tag="fac", name="fac")
            rows = drec[J, hpair]
            qs[3].dma_start(
                fac[:],
                bass.AP(tensor=rows.tensor, offset=rows.offset,
                        ap=[[512, 2], [0, 64], [1, 512]]))
            return uA, uhi, fac

        def emit_norm2(J, hpair, uA, uhi, fac):
            nc.vector.reciprocal(fac[:], fac[:])
            if DEBUG:
                nc.sync.dma_start(dbg_ua[J, hpair], uA[:])
                nc.sync.dma_start(dbg_uh[J, hpair], uhi[:])
                nc.sync.dma_start(dbg_fac[J, hpair], fac[:])
            nc.vector.tensor_mul(AT[0:64, hpair, J, :], uA[0:64, :], fac[0:64, :])
            nc.vector.tensor_mul(AT[64:128, hpair, J, :], uhi[64:128, :],
                                 fac[64:128, :])

        # ---------------- woven emission ----------------
        fq = deque()
        for tt in range(NT):
            fq.append(lambda tt=tt: emit_ln(tt))
            fq.append(lambda tt=tt: emit_q(tt))
            if tt - 2 >= 0:
                fq.append(lambda tt=tt: emit_k(tt - 2))
            if tt - 4 >= 0:
                fq.append(lambda tt=tt: emit_v(tt - 4))
        for tt in range(NT - 2, NT):
            fq.append(lambda tt=tt: emit_k(tt))
        for tt in range(NT - 4, NT):
            fq.append(lambda tt=tt: emit_v(tt))

        emitted = [0]

        def filler(n=1):
            for _ in range(n):
                if fq:
                    fq.popleft()()
                    emitted[0] += 1

        # qkv-unit watermark required before att(J): v(4J+3) must be emitted.
        NEED = [26, 42, 58, 64]
        # per-kt filler modulus per J (sparser late so units survive for the
        # J3 norm tail)
        FRATE = [3, 3, 4, 5]

        pending2 = None          # deferred norm2 args
        pending_out = None       # J whose out chains go to fq next
        for J in range(NJ):
            nkt = 4 * J + 4
            filler(max(0, NEED[J] - emitted[0]))
            for hpair in range(NP):
                filler(1)
                poA = pop.tile([65, 512], F32, tag="po", name="poA")
                poB = pop.tile([65, 512], F32, tag="po", name="poB")
                for kt in range(nkt):
                    emit_scores_tile()
                    emit_att_kt(J, hpair, kt, nkt, poA, poB)
                    if kt == 1 and pending2 is not None:
                        emit_norm2(*pending2)
                        pending2 = None
                        if pending_out is not None:
                            Jo = pending_out
                            for tc4 in range(4):
                                for ob in range(2):
                                    fq.append(lambda Jo=Jo, tc4=tc4, ob=ob:
                                              emit_out(Jo, tc4, ob))
                            pending_out = None
                    if kt % FRATE[J] == FRATE[J] - 1:
                        filler(1)
                n1 = emit_norm1(J, hpair, poA, poB)
                if pending2 is not None:
                    emit_norm2(*pending2)
                pending2 = (J, hpair) + n1
            pending_out = J
        emit_norm2(*pending2)
        for tc4 in range(4):
            for ob in range(2):
                fq.append(lambda tc4=tc4, ob=ob: emit_out(NJ - 1, tc4, ob))
        filler(len(fq))
        if DEBUG:
            nc.sync.dma_start(dbg_q[:], qT[:])
            nc.sync.dma_start(dbg_k[:], kT[:])
            nc.sync.dma_start(dbg_v[:], v_sb[:])
            nc.sync.dma_start(dbg_at[:], AT[:])

        for p in (drp, pop, spp, pap, outp, facp, uhp, uap, ptp, stp, htp, hp, xp, cst):
            p.release()
    nc.compile()
    return nc


def kernel(x, gamma, beta, w_qkv, w_out):
    x = np.asarray(x, dtype=np.float32)
    gamma = np.asarray(gamma, dtype=np.float32)
    beta = np.asarray(beta, dtype=np.float32)
    w_qkv = np.asarray(w_qkv, dtype=np.float32)
    w_out = np.asarray(w_out, dtype=np.float32)
    B = x.shape[0]
    beta_nonzero = bool(np.any(beta != 0.0))
    key = ("k", beta_nonzero)
    if key not in _CACHE:
        _CACHE[key] = _build(beta_nonzero)
    nc = _CACHE[key]

    i128, j128 = np.indices((128, 128))
    mask = np.where(i128 > j128, 0.0, 1.0).astype(ml_dtypes.bfloat16)
    ident = np.eye(128, dtype=ml_dtypes.bfloat16)
    betab = beta.reshape(1, C)

    in_maps = []
    for core in range(8):
        b, g = core // 2, core % 2
        sl = slice(g * 512, (g + 1) * 512)
        wq = (w_qkv[0 * C:1 * C][sl] * gamma[None, :]).T.copy()      # [1024, 512]
        wk = (w_qkv[1 * C:2 * C][sl] * gamma[None, :]).T.copy()
        wv = (w_qkv[2 * C:3 * C][sl] * gamma[None, :]).T.copy()
        wo = w_out[:, sl].T.copy()                                    # [512, 1024]
        in_maps.append({
            "x": np.ascontiguousarray(x[b]),
            "wq": wq.reshape(KC, 128, 512).astype(ml_dtypes.bfloat16),
            "wk": wk.reshape(KC, 128, 512).astype(ml_dtypes.bfloat16),
            "wv": wv.reshape(KC, 128, 512).astype(ml_dtypes.bfloat16),
            "wo": wo.reshape(NP, 128, 1024).astype(ml_dtypes.bfloat16),
            "masks": mask,
            "ident": ident,
            "betab": betab,
        })
    res = run_bass_kernel_spmd(nc, in_maps, core_ids=list(range(8)))
    out = np.empty((B, T, C), dtype=np.float32)
    for b in range(B):
        out[b] = res.results[2 * b]["out"] + res.results[2 * b + 1]["out"]
    return out


# revision 31
# speedup vs baseline: 1.1448x; 1.0143x over previous
"""CausalSelfAttention TRN2 kernel: LN + QKV + causal attention + out_proj.

Sharding: 8 cores = 4 batches x 2 head-groups (8 heads each). Each core
computes its batch's LayerNorm, QKV for its heads, causal softmax attention,
and a partial out-projection over its heads' channels; the host sums the two
partials per batch.

Schedule: single woven instruction stream. Per-tt LayerNorm/transpose/QKV
chains act as PE filler units interleaved into the attention J-blocks so the
PE never starves while ACT runs exp. Normalization avoids all HWDGE traffic:
sums ride the PV matmul (ones column in v), the B-half PSUM is moved to
partitions 63..127 by a gpsimd SWDGE DMA, reciprocal factors are broadcast
across partitions with gpsimd partition_broadcast, and out_proj DMAs its
PSUM tiles straight to DRAM.

Per-core layouts (SBUF partition dim first):
  hT   [c, t]   LN(x) transposed via PE, bf16, per-tt tiles
  qT/kT [o, t]  o = head*64+d; head pair (2i,2i+1) shares a 128-partition tile
  v    [t, (h, 65)] bf16, col 64 = ones (PV emits softmax sums as row 64)
  scores sp [tk, 2, tq] per kt tile (both pair halves share one PSUM tile so
  one ACT exp covers them); causal diagonal tiles slice rhs to [offs:512];
  causality inside the first 128 cols via a multiplicative [i>j] mask on DVE.
  out_proj: lhsT = AT [j, t] bf16, rhs = woT [j, o] bf16, PSUM -> DRAM DMA.
"""
import math
import sys
from collections import deque

sys.path.insert(0, "/opt/trn_rl_repo")
sys.path.insert(0, "/opt/trn_rl_repo/concourse")

import numpy as np
import ml_dtypes

import concourse.bass as bass
import concourse.bacc as bacc
import concourse.mybir as mybir
import concourse.tile as tile
from concourse.bass_utils import run_bass_kernel_spmd

T, C, NH, DH = 2048, 1024, 16, 64
HC = 8            # heads per core
NT = T // 128     # 16 t-tiles
KC = C // 128     # 8 contraction tiles
W = 512           # tq block width
NJ = T // W       # 4 q blocks
NP = HC // 2      # 4 head pairs
F32, BF16 = mybir.dt.float32, mybir.dt.bfloat16
AF = mybir.ActivationFunctionType
SUB, MUL = mybir.AluOpType.subtract, mybir.AluOpType.mult

_CACHE = {}
DEBUG = False


def _build(beta_nonzero):
    nc = bacc.Bacc("TRN2", target_bir_lowering=False, debug=False)
    dx = nc.dram_tensor("x", [T, C], F32, kind="ExternalInput")
    dwq = nc.dram_tensor("wq", [KC, 128, 512], BF16, kind="ExternalInput")
    dwk = nc.dram_tensor("wk", [KC, 128, 512], BF16, kind="ExternalInput")
    dwv = nc.dram_tensor("wv", [KC, 128, 512], BF16, kind="ExternalInput")
    dwo = nc.dram_tensor("wo", [NP, 128, 1024], BF16, kind="ExternalInput")
    dmask = nc.dram_tensor("masks", [128, 128], BF16, kind="ExternalInput")
    did = nc.dram_tensor("ident", [128, 128], BF16, kind="ExternalInput")
    dbeta = nc.dram_tensor("betab", [1, C], F32, kind="ExternalInput")
    dout = nc.dram_tensor("out", [T, C], F32, kind="ExternalOutput")
    if DEBUG:
        dbg_q = nc.dram_tensor("dbg_q", [128, NP, T], BF16, kind="ExternalOutput")
        dbg_k = nc.dram_tensor("dbg_k", [128, NP, T], BF16, kind="ExternalOutput")
        dbg_v = nc.dram_tensor("dbg_v", [128, NT, HC, 65], BF16, kind="ExternalOutput")
        dbg_at = nc.dram_tensor("dbg_at", [128, NP, NJ, 512], BF16, kind="ExternalOutput")
        dbg_ua = nc.dram_tensor("dbg_ua", [NJ, NP, 65, 512], F32, kind="ExternalOutput")
        dbg_uh = nc.dram_tensor("dbg_uh", [NJ, NP, 128, 512], F32, kind="ExternalOutput")
        dbg_fac = nc.dram_tensor("dbg_fac", [NJ, NP, 128, 512], F32, kind="ExternalOutput")

    with tile.TileContext(nc) as tc:
        cst = tc.alloc_tile_pool(name="cst", bufs=1)
        ident = cst.tile([128, 128], BF16)
        mask1 = cst.tile([128, 128], BF16)
        eps = cst.tile([128, 1], F32)
        wq_sb = cst.tile([128, KC, 512], BF16)
        wk_sb = cst.tile([128, KC, 512], BF16)
        wv_sb = cst.tile([128, KC, 512], BF16)
        wo_sb = cst.tile([128, NP, 1024], BF16)
        qT = cst.tile([128, NP, T], BF16)
        kT = cst.tile([128, NP, T], BF16)
        v_sb = cst.tile([128, NT, HC, 65], BF16)
        AT = cst.tile([128, NP, NJ, 512], BF16)
        if beta_nonzero:
            beta_sb = cst.tile([128, C], F32)

        ones64 = cst.tile([128, 64], F32)
        nc.vector.memset(eps[:], 1e-5)
        nc.vector.memset(v_sb[:, :, :, 64:65], 1.0)
        nc.vector.memset(ones64[:], 1.0)

        xp = tc.alloc_tile_pool(name="xp", bufs=6)
        hp = tc.alloc_tile_pool(name="hp", bufs=3)
        htp = tc.alloc_tile_pool(name="htp", bufs=8)
        stp = tc.alloc_tile_pool(name="stp", bufs=4)
        ptp = tc.alloc_tile_pool(name="ptp", bufs=6)
        uap = tc.alloc_tile_pool(name="uap", bufs=6)
        uhp = tc.alloc_tile_pool(name="uhp", bufs=4)
        facp = tc.alloc_tile_pool(name="facp", bufs=4)
        outp = tc.alloc_tile_pool(name="outp", bufs=4)
        pap = tc.alloc_tile_pool(name="pap", bufs=2, space="PSUM")
        spp = tc.alloc_tile_pool(name="spp", bufs=2, space="PSUM")
        pop = tc.alloc_tile_pool(name="pop", bufs=2, space="PSUM")
        drp = tc.alloc_tile_pool(name="drp", bufs=1, space="DRAM")
        drec = drp.tile([NJ, NP, 2, 512], F32)

        # ---- DMA issue: sync/HWDGE queue (x0-1, weights, x8-15, wo)
        xts = []
        for tt in range(NT):
            xt = xp.tile([128, C], F32, tag="x", name=f"xt{tt}")
            xts.append(xt)
        # ident on the Pool queue first (first transposes need it ~6us);
        # x0/x1 on sync (earliest arrival), x2-7 on Pool/SWDGE. Writers must
        # appear in allocation order per slot, so keep tt order.
        nc.gpsimd.dma_start(ident[:], did[:])
        nc.sync.dma_start(xts[0][:], dx[0:128, :])
        nc.sync.dma_start(xts[1][:], dx[128:256, :])
        for tt in range(2, 8):
            nc.gpsimd.dma_start(xts[tt][:], dx[tt * 128:(tt + 1) * 128, :])
        for kc in range(KC):
            nc.sync.dma_start(wq_sb[:, kc, :], dwq[kc])
        for kc in range(KC):
            nc.sync.dma_start(wk_sb[:, kc, :], dwk[kc])
        nc.sync.dma_start(mask1[:], dmask[:])
        for kc in range(KC):
            nc.sync.dma_start(wv_sb[:, kc, :], dwv[kc])
        for tt in range(8, NT):
            nc.sync.dma_start(xts[tt][:], dx[tt * 128:(tt + 1) * 128, :])
        for jp in range(NP):
            nc.sync.dma_start(wo_sb[:, jp, :], dwo[jp])
        if beta_nonzero:
            bap = dbeta[0:1, :]
            nc.sync.dma_start(
                beta_sb[:],
                bass.AP(tensor=bap.tensor, offset=bap.offset,
                        ap=[[0, 128], bap.ap[1]]))

        hTs = [None] * NT

        def emit_ln(tt):
            xt = xts[tt]
            stats = stp.tile([128, 2, 6], F32, tag="st")
            xg = xt[:].rearrange("p (g d) -> p g d", g=2)
            for g in range(2):
                nc.vector.bn_stats(stats[:, g, :], xg[:, g, :])
            mv = stp.tile([128, 2], F32, tag="mv")
            nc.vector.bn_aggr(mv[:], stats[:])
            sd = stp.tile([128, 1], F32, tag="sd")
            nc.scalar.activation(sd[:], mv[:, 1:2], AF.Sqrt, bias=eps[:], scale=1.0)
            nc.vector.reciprocal(sd[:], sd[:])
            ht = hp.tile([128, C], BF16, tag="h")
            nc.vector.tensor_scalar(
                out=ht[:], in0=xt[:], scalar1=mv[:, 0:1], scalar2=sd[:],
                op0=SUB, op1=MUL)
            if beta_nonzero:
                nc.vector.tensor_add(ht[:], ht[:], beta_sb[:])
            tp = pap.tile([128, KC, 128], BF16, tag="pa", name="tp")
            for kc in range(KC):
                nc.tensor.transpose(tp[:, kc, :], ht[:, kc * 128:(kc + 1) * 128], ident[:])
            hT = htp.tile([128, KC, 128], BF16, tag="hT", name=f"hT{tt}")
            nc.vector.tensor_copy(hT[:], tp[:])
            hTs[tt] = hT

        def emit_q(tt):
            q4 = pap.tile([128, NP, 128], F32, tag="pa", name="q4")
            for ot in range(NP):
                for kc in range(KC):
                    nc.tensor.matmul(q4[:, ot, :],
                                     wq_sb[:, kc, ot * 128:(ot + 1) * 128],
                                     hTs[tt][:, kc, :],
                                     start=(kc == 0), stop=(kc == KC - 1))
            nc.vector.tensor_copy(qT[:, :, tt * 128:(tt + 1) * 128], q4[:])

        def emit_k(tt):
            k4 = pap.tile([128, NP, 128], F32, tag="pa", name="k4")
            for ot in range(NP):
                for kc in range(KC):
                    nc.tensor.matmul(k4[:, ot, :],
                                     wk_sb[:, kc, ot * 128:(ot + 1) * 128],
                                     hTs[tt][:, kc, :],
                                     start=(kc == 0), stop=(kc == KC - 1))
            nc.vector.tensor_copy(kT[:, :, tt * 128:(tt + 1) * 128], k4[:])

        def emit_v(tt):
            v1 = pap.tile([128, 512], F32, tag="pa", name="v1")
            for kc in range(KC):
                nc.tensor.matmul(v1[:], hTs[tt][:, kc, :], wv_sb[:, kc, :],
                                 start=(kc == 0), stop=(kc == KC - 1))
            nc.vector.tensor_copy(
                v_sb[:, tt, :, 0:64],
                v1[:].rearrange("p (h d) -> p h d", h=HC))

        def emit_out(J, tc4, ob):
            pp = pap.tile([128, 512], F32, tag="pa", name="pp")
            for hp_ in range(NP):
                nc.tensor.matmul(pp[:],
                                 AT[:, hp_, J, tc4 * 128:(tc4 + 1) * 128],
                                 wo_sb[:, hp_, ob * 512:(ob + 1) * 512],
                                 start=(hp_ == 0), stop=(hp_ == NP - 1))
            ot_ = outp.tile([128, 512], F32, tag="ot", name="ot")
            nc.vector.tensor_copy(ot_[:], pp[:])
            t0 = J * 512 + tc4 * 128
            nc.sync.dma_start(dout[t0:t0 + 128, ob * 512:(ob + 1) * 512], ot_[:])

        # ---------------- attention ----------------
        def emit_scores_exp(J, hpair, kt):
            offs = max(0, (kt - 4 * J) * 128)
            sp = spp.tile([128, 2, 512], F32, tag="sp", name="sp")
            for half, base in ((0, 0), (1, 64)):
                nc.tensor.matmul(
                    sp[:, half, offs:512],
                    kT[base:base + 64, hpair, kt * 128:(kt + 1) * 128],
                    qT[base:base + 64, hpair, J * 512 + offs:(J + 1) * 512],
                    start=True, stop=True,
                    tile_position=(base, 0))
            pt = ptp.tile([128, 2, 512], BF16, tag="pt", name="pt")
            if offs == 0:
                nc.scalar.activation(
                    pt[:].rearrange("p g f -> p (g f)"),
                    sp[:].rearrange("p g f -> p (g f)"),
                    AF.Exp, scale=0.125)
            else:
                nc.scalar.activation(pt[:, :, offs:512], sp[:, :, offs:512],
                                     AF.Exp, scale=0.125)
            if kt - 4 * J >= 0:
                r = offs
                for half in range(2):
                    nc.vector.tensor_mul(pt[:, half, r:r + 128],
                                         pt[:, half, r:r + 128],
                                         mask1[:, 0:128])
            return pt, offs

        def emit_pv(kt, nkt, poA, poB, hpair, pt, offs):
            for half, po in ((0, poA), (1, poB)):
                h = 2 * hpair + half
                nc.tensor.matmul(po[:, offs:512], v_sb[:, kt, h, :],
                                 pt[:, half, offs:512],
                                 start=(kt == 0), stop=(kt == nkt - 1))

        def emit_norm1(J, hpair, poA, poB):
            # u copies (DVE, PSUM->SBUF), B-half U rows -> partitions 64..127
            # (gpsimd SWDGE SBUF->SBUF), sums-row broadcasts (gpsimd) into fac.
            uA = uap.tile([65, 512], F32, tag="ua", name="uA")
            nc.vector.tensor_copy(uA[:], poA[:])
            uB = uap.tile([65, 512], F32, tag="ua", name="uB")
            nc.vector.tensor_copy(uB[:], poB[:])
            uhi = uhp.tile([128, 512], F32, tag="uh", name="uhi")
            last = (J == NJ - 1 and hpair == NP - 1)
            if last:
                # Tail: low-latency variant. uhi via sync HWDGE; reciprocal
                # factors via a PE ones-outer-product into the just-freed po
                # PSUM slot (no DRAM roundtrip).
                nc.sync.dma_start(uhi[64:128, :], uB[0:64, :])
                fac = pop.tile([128, 512], F32, tag="po", name="facp_")
                nc.tensor.matmul(fac[0:64, :], ones64[64:65, :], uA[64:65, :],
                                 start=True, stop=True)
                nc.tensor.matmul(fac[64:128, :], ones64[64:65, :], uB[64:65, :],
                                 start=True, stop=True)
                return uA, uhi, fac
            nc.gpsimd.dma_start(uhi[64:128, :], uB[0:64, :])
            # sums rows -> DRAM, then one broadcast load into fac (SBUF src
            # partition dim cannot have stride 0; DRAM src can repeat).
            nc.gpsimd.dma_start(drec[J, hpair, 0:1, :], uA[64:65, :])
            nc.gpsimd.dma_start(drec[J, hpair, 1:2, :], uB[64:65, :])
            fac = facp.tile([128, 512], F32, tag="fac", name="fac")
            rows = drec[J, hpair]
            nc.gpsimd.dma_start(
                fac[:],
                bass.AP(tensor=rows.tensor, offset=rows.offset,
                        ap=[[512, 2], [0, 64], [1, 512]]))
            return uA, uhi, fac

        def emit_norm2(J, hpair, uA, uhi, fac):
            nc.vector.reciprocal(fac[:], fac[:])
            if DEBUG:
                nc.sync.dma_start(dbg_ua[J, hpair], uA[:])
                nc.sync.dma_start(dbg_uh[J, hpair], uhi[:])
                nc.sync.dma_start(dbg_fac[J, hpair], fac[:])
            nc.vector.tensor_mul(AT[0:64, hpair, J, :], uA[0:64, :], fac[0:64, :])
            nc.vector.tensor_mul(AT[64:128, hpair, J, :], uhi[64:128, :],
                                 fac[64:128, :])

        # ---------------- woven emission ----------------
        fq = deque()
        for tt in range(NT):
            fq.append(lambda tt=tt: emit_ln(tt))
            fq.append(lambda tt=tt: emit_q(tt))
            if tt - 2 >= 0:
                fq.append(lambda tt=tt: emit_k(tt - 2))
            if tt - 4 >= 0:
                fq.append(lambda tt=tt: emit_v(tt - 4))
        for tt in range(NT - 2, NT):
            fq.append(lambda tt=tt: emit_k(tt))
        for tt in range(NT - 4, NT):
            fq.append(lambda tt=tt: emit_v(tt))

        emitted = [0]

        def filler(n=1):
            for _ in range(n):
                if fq:
                    fq.popleft()()
                    emitted[0] += 1

        # qkv-unit watermark required before att(J): v(4J+3) must be emitted.
        NEED = [26, 42, 58, 64]
        # per-kt filler modulus per J (sparser late so units survive for the
        # J3 norm tail)
        FRATE = [3, 3, 4, 5]

        pending2 = None          # deferred norm2 args
        pending_out = None       # J whose out chains go to fq next
        for J in range(NJ):
            nkt = 4 * J + 4
            filler(max(0, NEED[J] - emitted[0]))
            for hpair in range(NP):
                filler(1)
                poA = pop.tile([65, 512], F32, tag="po", name="poA")
                poB = pop.tile([65, 512], F32, tag="po", name="poB")
                prev = None
                for kt in range(nkt):
                    cur = emit_scores_exp(J, hpair, kt)
                    if prev is not None:
                        emit_pv(kt - 1, nkt, poA, poB, hpair, *prev)
                    prev = cur
                    if kt == 1 and pending2 is not None:
                        emit_norm2(*pending2)
                        pending2 = None
                        if pending_out is not None:
                            Jo = pending_out
                            for tc4 in range(4):
                                for ob in range(2):
                                    fq.append(lambda Jo=Jo, tc4=tc4, ob=ob:
                                              emit_out(Jo, tc4, ob))
                            pending_out = None
                    if kt % FRATE[J] == FRATE[J] - 1:
                        filler(1)
                filler(1)
                emit_pv(nkt - 1, nkt, poA, poB, hpair, *prev)
                n1 = emit_norm1(J, hpair, poA, poB)
                if pending2 is not None:
                    emit_norm2(*pending2)
                pending2 = (J, hpair) + n1
            pending_out = J
        emit_norm2(*pending2)
        for tc4 in range(4):
            for ob in range(2):
                fq.append(lambda tc4=tc4, ob=ob: emit_out(NJ - 1, tc4, ob))
        filler(len(fq))
        if DEBUG:
            nc.sync.dma_start(dbg_q[:], qT[:])
            nc.sync.dma_start(dbg_k[:], kT[:])
            nc.sync.dma_start(dbg_v[:], v_sb[:])
            nc.sync.dma_start(dbg_at[:], AT[:])

        for p in (drp, pop, spp, pap, outp, facp, uhp, uap, ptp, stp, htp, hp, xp, cst):
            p.release()
    nc.compile()
    return nc


def kernel(x, gamma, beta, w_qkv, w_out):
    x = np.asarray(x, dtype=np.float32)
    gamma = np.asarray(gamma, dtype=np.float32)
    beta = np.asarray(beta, dtype=np.float32)
    w_qkv = np.asarray(w_qkv, dtype=np.float32)
    w_out = np.asarray(w_out, dtype=np.float32)
    B = x.shape[0]
    beta_nonzero = bool(np.any(beta != 0.0))
    key = ("k", beta_nonzero)
    if key not in _CACHE:
        _CACHE[key] = _build(beta_nonzero)
    nc = _CACHE[key]

    i128, j128 = np.indices((128, 128))
    mask = np.where(i128 > j128, 0.0, 1.0).astype(ml_dtypes.bfloat16)
    ident = np.eye(128, dtype=ml_dtypes.bfloat16)
    betab = beta.reshape(1, C)

    in_maps = []
    for core in range(8):
        b, g = core // 2, core % 2
        sl = slice(g * 512, (g + 1) * 512)
        wq = (w_qkv[0 * C:1 * C][sl] * gamma[None, :]).T.copy()      # [1024, 512]
        wk = (w_qkv[1 * C:2 * C][sl] * gamma[None, :]).T.copy()
        wv = (w_qkv[2 * C:3 * C][sl] * gamma[None, :]).T.copy()
        wo = w_out[:, sl].T.copy()                                    # [512, 1024]
        in_maps.append({
            "x": np.ascontiguousarray(x[b]),
            "wq": wq.reshape(KC, 128, 512).astype(ml_dtypes.bfloat16),
            "wk": wk.reshape(KC, 128, 512).astype(ml_dtypes.bfloat16),
            "wv": wv.reshape(KC, 128, 512).astype(ml_dtypes.bfloat16),
            "wo": wo.reshape(NP, 128, 1024).astype(ml_dtypes.bfloat16),
            "masks": mask,
            "ident": ident,
            "betab": betab,
        })
    res = run_bass_kernel_spmd(nc, in_maps, core_ids=list(range(8)))
    out = np.empty((B, T, C), dtype=np.float32)
    for b in range(B):
        out[b] = res.results[2 * b]["out"] + res.results[2 * b + 1]["out"]
    return out


# revision 34
# speedup vs baseline: 1.1468x; 1.0018x over previous
"""CausalSelfAttention TRN2 kernel: LN + QKV + causal attention + out_proj.

Sharding: 8 cores = 4 batches x 2 head-groups (8 heads each). Each core
computes its batch's LayerNorm, QKV for its heads, causal softmax attention,
and a partial out-projection over its heads' channels; the host sums the two
partials per batch.

Schedule: single woven instruction stream. Per-tt LayerNorm/transpose/QKV
chains act as PE filler units interleaved into the attention J-blocks so the
PE never starves while ACT runs exp. Normalization avoids all HWDGE traffic:
sums ride the PV matmul (ones column in v), the B-half PSUM is moved to
partitions 63..127 by a gpsimd SWDGE DMA, reciprocal factors are broadcast
across partitions with gpsimd partition_broadcast, and out_proj DMAs its
PSUM tiles straight to DRAM.

Per-core layouts (SBUF partition dim first):
  hT   [c, t]   LN(x) transposed via PE, bf16, per-tt tiles
  qT/kT [o, t]  o = head*64+d; head pair (2i,2i+1) shares a 128-partition tile
  v    [t, (h, 65)] bf16, col 64 = ones (PV emits softmax sums as row 64)
  scores sp [tk, 2, tq] per kt tile (both pair halves share one PSUM tile so
  one ACT exp covers them); causal diagonal tiles slice rhs to [offs:512];
  causality inside the first 128 cols via a multiplicative [i>j] mask on DVE.
  out_proj: lhsT = AT [j, t] bf16, rhs = woT [j, o] bf16, PSUM -> DRAM DMA.
"""
import math
import sys
from collections import deque

sys.path.insert(0, "/opt/trn_rl_repo")
sys.path.insert(0, "/opt/trn_rl_repo/concourse")

import numpy as np
import ml_dtypes

import concourse.bass as bass
import concourse.bacc as bacc
import concourse.mybir as mybir
import concourse.tile as tile
from concourse.bass_utils import run_bass_kernel_spmd

T, C, NH, DH = 2048, 1024, 16, 64
HC = 8            # heads per core
NT = T // 128     # 16 t-tiles
KC = C // 128     # 8 contraction tiles
W = 512           # tq block width
NJ = T // W       # 4 q blocks
NP = HC // 2      # 4 head pairs
F32, BF16 = mybir.dt.float32, mybir.dt.bfloat16
AF = mybir.ActivationFunctionType
SUB, MUL = mybir.AluOpType.subtract, mybir.AluOpType.mult

_CACHE = {}
DEBUG = False


def _build(beta_nonzero):
    nc = bacc.Bacc("TRN2", target_bir_lowering=False, debug=False)
    dx = nc.dram_tensor("x", [T, C], F32, kind="ExternalInput")
    dwq = nc.dram_tensor("wq", [KC, 128, 512], BF16, kind="ExternalInput")
    dwk = nc.dram_tensor("wk", [KC, 128, 512], BF16, kind="ExternalInput")
    dwv = nc.dram_tensor("wv", [KC, 128, 512], BF16, kind="ExternalInput")
    dwo = nc.dram_tensor("wo", [NP, 128, 1024], BF16, kind="ExternalInput")
    dmask = nc.dram_tensor("masks", [128, 128], BF16, kind="ExternalInput")
    did = nc.dram_tensor("ident", [128, 128], BF16, kind="ExternalInput")
    dbeta = nc.dram_tensor("betab", [1, C], F32, kind="ExternalInput")
    dout = nc.dram_tensor("out", [T, C], F32, kind="ExternalOutput")
    if DEBUG:
        dbg_q = nc.dram_tensor("dbg_q", [128, NP, T], BF16, kind="ExternalOutput")
        dbg_k = nc.dram_tensor("dbg_k", [128, NP, T], BF16, kind="ExternalOutput")
        dbg_v = nc.dram_tensor("dbg_v", [128, NT, HC, 65], BF16, kind="ExternalOutput")
        dbg_at = nc.dram_tensor("dbg_at", [128, NP, NJ, 512], BF16, kind="ExternalOutput")
        dbg_ua = nc.dram_tensor("dbg_ua", [NJ, NP, 65, 512], F32, kind="ExternalOutput")
        dbg_uh = nc.dram_tensor("dbg_uh", [NJ, NP, 128, 512], F32, kind="ExternalOutput")
        dbg_fac = nc.dram_tensor("dbg_fac", [NJ, NP, 128, 512], F32, kind="ExternalOutput")

    with tile.TileContext(nc) as tc:
        cst = tc.alloc_tile_pool(name="cst", bufs=1)
        ident = cst.tile([128, 128], BF16)
        mask1 = cst.tile([128, 128], BF16)
        eps = cst.tile([128, 1], F32)
        wq_sb = cst.tile([128, KC, 512], BF16)
        wk_sb = cst.tile([128, KC, 512], BF16)
        wv_sb = cst.tile([128, KC, 512], BF16)
        wo_sb = cst.tile([128, NP, 1024], BF16)
        qT = cst.tile([128, NP, T], BF16)
        kT = cst.tile([128, NP, T], BF16)
        v_sb = cst.tile([128, NT, HC, 65], BF16)
        AT = cst.tile([128, NP, NJ, 512], BF16)
        if beta_nonzero:
            beta_sb = cst.tile([128, C], F32)

        ones64 = cst.tile([128, 64], F32)

        xp = tc.alloc_tile_pool(name="xp", bufs=6)
        hp = tc.alloc_tile_pool(name="hp", bufs=3)
        htp = tc.alloc_tile_pool(name="htp", bufs=8)
        stp = tc.alloc_tile_pool(name="stp", bufs=4)
        ptp = tc.alloc_tile_pool(name="ptp", bufs=6)
        uap = tc.alloc_tile_pool(name="uap", bufs=6)
        uhp = tc.alloc_tile_pool(name="uhp", bufs=4)
        facp = tc.alloc_tile_pool(name="facp", bufs=4)
        outp = tc.alloc_tile_pool(name="outp", bufs=4)
        pap = tc.alloc_tile_pool(name="pap", bufs=2, space="PSUM")
        spp = tc.alloc_tile_pool(name="spp", bufs=2, space="PSUM")
        pop = tc.alloc_tile_pool(name="pop", bufs=2, space="PSUM")
        drp = tc.alloc_tile_pool(name="drp", bufs=1, space="DRAM")
        drec = drp.tile([NJ, NP, 2, 512], F32)

        nc.vector.memset(eps[:], 1e-5)
        nc.vector.memset(v_sb[:, :, :, 64:65], 1.0)
        nc.vector.memset(ones64[:], 1.0)

        # ---- DMA issue: sync/HWDGE queue (x0-1, weights, x8-15, wo)
        xts = []
        for tt in range(NT):
            xt = xp.tile([128, C], F32, tag="x", name=f"xt{tt}")
            xts.append(xt)
        # ident on the Pool queue first (first transposes need it ~6us);
        # x0/x1 on sync (earliest arrival), x2-7 on Pool/SWDGE. Writers must
        # appear in allocation order per slot, so keep tt order.
        nc.gpsimd.dma_start(ident[:], did[:])
        nc.sync.dma_start(xts[0][:], dx[0:128, :])
        nc.sync.dma_start(xts[1][:], dx[128:256, :])
        for tt in range(2, 8):
            nc.gpsimd.dma_start(xts[tt][:], dx[tt * 128:(tt + 1) * 128, :])
        for kc in range(KC):
            nc.sync.dma_start(wq_sb[:, kc, :], dwq[kc])
        for kc in range(KC):
            nc.sync.dma_start(wk_sb[:, kc, :], dwk[kc])
        nc.sync.dma_start(mask1[:], dmask[:])
        for kc in range(KC):
            nc.sync.dma_start(wv_sb[:, kc, :], dwv[kc])
        for tt in range(8, NT):
            nc.sync.dma_start(xts[tt][:], dx[tt * 128:(tt + 1) * 128, :])
        for jp in range(NP):
            nc.sync.dma_start(wo_sb[:, jp, :], dwo[jp])
        if beta_nonzero:
            bap = dbeta[0:1, :]
            nc.sync.dma_start(
                beta_sb[:],
                bass.AP(tensor=bap.tensor, offset=bap.offset,
                        ap=[[0, 128], bap.ap[1]]))

        hTs = [None] * NT

        def emit_ln(tt):
            xt = xts[tt]
            stats = stp.tile([128, 2, 6], F32, tag="st")
            xg = xt[:].rearrange("p (g d) -> p g d", g=2)
            for g in range(2):
                nc.vector.bn_stats(stats[:, g, :], xg[:, g, :])
            mv = stp.tile([128, 2], F32, tag="mv")
            nc.vector.bn_aggr(mv[:], stats[:])
            sd = stp.tile([128, 1], F32, tag="sd")
            nc.scalar.activation(sd[:], mv[:, 1:2], AF.Sqrt, bias=eps[:], scale=1.0)
            nc.vector.reciprocal(sd[:], sd[:])
            ht = hp.tile([128, C], BF16, tag="h")
            nc.vector.tensor_scalar(
                out=ht[:], in0=xt[:], scalar1=mv[:, 0:1], scalar2=sd[:],
                op0=SUB, op1=MUL)
            if beta_nonzero:
                nc.vector.tensor_add(ht[:], ht[:], beta_sb[:])
            tp = pap.tile([128, KC, 128], BF16, tag="pa", name="tp")
            for kc in range(KC):
                nc.tensor.transpose(tp[:, kc, :], ht[:, kc * 128:(kc + 1) * 128], ident[:])
            hT = htp.tile([128, KC, 128], BF16, tag="hT", name=f"hT{tt}")
            nc.vector.tensor_copy(hT[:], tp[:])
            hTs[tt] = hT

        def emit_q(tt):
            q4 = pap.tile([128, NP, 128], F32, tag="pa", name="q4")
            for ot in range(NP):
                for kc in range(KC):
                    nc.tensor.matmul(q4[:, ot, :],
                                     wq_sb[:, kc, ot * 128:(ot + 1) * 128],
                                     hTs[tt][:, kc, :],
                                     start=(kc == 0), stop=(kc == KC - 1))
            nc.vector.tensor_copy(qT[:, :, tt * 128:(tt + 1) * 128], q4[:])

        def emit_k(tt):
            k4 = pap.tile([128, NP, 128], F32, tag="pa", name="k4")
            for ot in range(NP):
                for kc in range(KC):
                    nc.tensor.matmul(k4[:, ot, :],
                                     wk_sb[:, kc, ot * 128:(ot + 1) * 128],
                                     hTs[tt][:, kc, :],
                                     start=(kc == 0), stop=(kc == KC - 1))
            nc.vector.tensor_copy(kT[:, :, tt * 128:(tt + 1) * 128], k4[:])

        def emit_v(tt):
            v1 = pap.tile([128, 512], F32, tag="pa", name="v1")
            for kc in range(KC):
                nc.tensor.matmul(v1[:], hTs[tt][:, kc, :], wv_sb[:, kc, :],
                                 start=(kc == 0), stop=(kc == KC - 1))
            nc.vector.tensor_copy(
                v_sb[:, tt, :, 0:64],
                v1[:].rearrange("p (h d) -> p h d", h=HC))

        def emit_out(J, tc4, ob):
            pp = pap.tile([128, 512], F32, tag="pa", name="pp")
            for hp_ in range(NP):
                nc.tensor.matmul(pp[:],
                                 AT[:, hp_, J, tc4 * 128:(tc4 + 1) * 128],
                                 wo_sb[:, hp_, ob * 512:(ob + 1) * 512],
                                 start=(hp_ == 0), stop=(hp_ == NP - 1))
            ot_ = outp.tile([128, 512], F32, tag="ot", name="ot")
            nc.vector.tensor_copy(ot_[:], pp[:])
            t0 = J * 512 + tc4 * 128
            nc.sync.dma_start(dout[t0:t0 + 128, ob * 512:(ob + 1) * 512], ot_[:])

        # ---------------- attention ----------------
        def emit_scores_exp(J, hpair, kt):
            offs = max(0, (kt - 4 * J) * 128)
            sp = spp.tile([128, 2, 512], F32, tag="sp", name="sp")
            for half, base in ((0, 0), (1, 64)):
                nc.tensor.matmul(
                    sp[:, half, offs:512],
                    kT[base:base + 64, hpair, kt * 128:(kt + 1) * 128],
                    qT[base:base + 64, hpair, J * 512 + offs:(J + 1) * 512],
                    start=True, stop=True,
                    tile_position=(base, 0))
            pt = ptp.tile([128, 2, 512], BF16, tag="pt", name="pt")
            if offs == 0:
                nc.scalar.activation(
                    pt[:].rearrange("p g f -> p (g f)"),
                    sp[:].rearrange("p g f -> p (g f)"),
                    AF.Exp, scale=0.125)
            else:
                nc.scalar.activation(pt[:, :, offs:512], sp[:, :, offs:512],
                                     AF.Exp, scale=0.125)
            if kt - 4 * J >= 0:
                r = offs
                for half in range(2):
                    nc.vector.tensor_mul(pt[:, half, r:r + 128],
                                         pt[:, half, r:r + 128],
                                         mask1[:, 0:128])
            return pt, offs

        def emit_pv(kt, nkt, poA, poB, hpair, pt, offs):
            for half, po in ((0, poA), (1, poB)):
                h = 2 * hpair + half
                nc.tensor.matmul(po[:, offs:512], v_sb[:, kt, h, :],
                                 pt[:, half, offs:512],
                                 start=(kt == 0), stop=(kt == nkt - 1))

        def emit_norm1(J, hpair, poA, poB):
            # u copies (DVE, PSUM->SBUF), B-half U rows -> partitions 64..127
            # (gpsimd SWDGE SBUF->SBUF), sums-row broadcasts (gpsimd) into fac.
            uA = uap.tile([65, 512], F32, tag="ua", name="uA")
            nc.vector.tensor_copy(uA[:], poA[:])
            uB = uap.tile([65, 512], F32, tag="ua", name="uB")
            nc.vector.tensor_copy(uB[:], poB[:])
            uhi = uhp.tile([128, 512], F32, tag="uh", name="uhi")
            last = (J == NJ - 1 and hpair == NP - 1)
            if last:
                # Tail: low-latency variant. uhi via sync HWDGE; reciprocal
                # factors via a PE ones-outer-product into the just-freed po
                # PSUM slot (no DRAM roundtrip).
                nc.sync.dma_start(uhi[64:128, :], uB[0:64, :])
                fac = pop.tile([128, 512], F32, tag="po", name="facp_")
                nc.tensor.matmul(fac[0:64, :], ones64[64:65, :], uA[64:65, :],
                                 start=True, stop=True)
                nc.tensor.matmul(fac[64:128, :], ones64[64:65, :], uB[64:65, :],
                                 start=True, stop=True)
                return uA, uhi, fac
            nc.gpsimd.dma_start(uhi[64:128, :], uB[0:64, :])
            # sums rows -> DRAM, then one broadcast load into fac (SBUF src
            # partition dim cannot have stride 0; DRAM src can repeat).
            nc.gpsimd.dma_start(drec[J, hpair, 0:1, :], uA[64:65, :])
            nc.gpsimd.dma_start(drec[J, hpair, 1:2, :], uB[64:65, :])
            fac = facp.tile([128, 512], F32, tag="fac", name="fac")
            rows = drec[J, hpair]
            nc.gpsimd.dma_start(
                fac[:],
                bass.AP(tensor=rows.tensor, offset=rows.offset,
                        ap=[[512, 2], [0, 64], [1, 512]]))
            return uA, uhi, fac

        def emit_norm2(J, hpair, uA, uhi, fac):
            nc.vector.reciprocal(fac[:], fac[:])
            if DEBUG:
                nc.sync.dma_start(dbg_ua[J, hpair], uA[:])
                nc.sync.dma_start(dbg_uh[J, hpair], uhi[:])
                nc.sync.dma_start(dbg_fac[J, hpair], fac[:])
            nc.vector.tensor_mul(AT[0:64, hpair, J, :], uA[0:64, :], fac[0:64, :])
            nc.vector.tensor_mul(AT[64:128, hpair, J, :], uhi[64:128, :],
                                 fac[64:128, :])

        # ---------------- woven emission ----------------
        fq = deque()
        for tt in range(NT):
            fq.append(lambda tt=tt: emit_ln(tt))
            fq.append(lambda tt=tt: emit_q(tt))
            if tt - 2 >= 0:
                fq.append(lambda tt=tt: emit_k(tt - 2))
            if tt - 4 >= 0:
                fq.append(lambda tt=tt: emit_v(tt - 4))
        for tt in range(NT - 2, NT):
            fq.append(lambda tt=tt: emit_k(tt))
        for tt in range(NT - 4, NT):
            fq.append(lambda tt=tt: emit_v(tt))

        emitted = [0]

        def filler(n=1):
            for _ in range(n):
                if fq:
                    fq.popleft()()
                    emitted[0] += 1

        # qkv-unit watermark required before att(J): v(4J+3) must be emitted.
        NEED = [26, 42, 58, 64]
        # per-kt filler modulus per J (sparser late so units survive for the
        # J3 norm tail)
        FRATE = [3, 3, 4, 5]

        pending2 = None          # deferred norm2 args
        pending_out = None       # J whose out chains go to fq next
        for J in range(NJ):
            nkt = 4 * J + 4
            filler(max(0, NEED[J] - emitted[0]))
            for hpair in range(NP):
                filler(1)
                poA = pop.tile([65, 512], F32, tag="po", name="poA")
                poB = pop.tile([65, 512], F32, tag="po", name="poB")
                prev = None
                for kt in range(nkt):
                    cur = emit_scores_exp(J, hpair, kt)
                    if prev is not None:
                        emit_pv(kt - 1, nkt, poA, poB, hpair, *prev)
                    prev = cur
                    if kt % FRATE[J] == FRATE[J] - 1:
                        filler(1)
                filler(1)
                emit_pv(nkt - 1, nkt, poA, poB, hpair, *prev)
                # deferred norm2 of the previous hpair: its fac DMA roundtrip
                # has had a full hpair to land, and this hpair's mask muls are
                # already ahead of it in DVE order.
                if pending2 is not None:
                    emit_norm2(*pending2)
                    pending2 = None
                    if pending_out is not None:
                        Jo = pending_out
                        for tc4 in range(4):
                            for ob in range(2):
                                fq.append(lambda Jo=Jo, tc4=tc4, ob=ob:
                                          emit_out(Jo, tc4, ob))
                        pending_out = None
                n1 = emit_norm1(J, hpair, poA, poB)
                pending2 = (J, hpair) + n1
            pending_out = J
        emit_norm2(*pending2)
        for tc4 in range(4):
            for ob in range(2):
                fq.append(lambda tc4=tc4, ob=ob: emit_out(NJ - 1, tc4, ob))
        filler(len(fq))
        if DEBUG:
            nc.sync.dma_start(dbg_q[:], qT[:])
            nc.sync.dma_start(dbg_k[:], kT[:])
            nc.sync.dma_start(dbg_v[:], v_sb[:])
            nc.sync.dma_start(dbg_at[:], AT[:])

        for p in (drp, pop, spp, pap, outp, facp, uhp, uap, ptp, stp, htp, hp, xp, cst):
            p.release()
    nc.compile()
    return nc


def kernel(x, gamma, beta, w_qkv, w_out):
    x = np.asarray(x, dtype=np.float32)
    gamma = np.asarray(gamma, dtype=np.float32)
    beta = np.asarray(beta, dtype=np.float32)
    w_qkv = np.asarray(w_qkv, dtype=np.float32)
    w_out = np.asarray(w_out, dtype=np.float32)
    B = x.shape[0]
    beta_nonzero = bool(np.any(beta != 0.0))
    key = ("k", beta_nonzero)
    if key not in _CACHE:
        _CACHE[key] = _build(beta_nonzero)
    nc = _CACHE[key]

    i128, j128 = np.indices((128, 128))
    mask = np.where(i128 > j128, 0.0, 1.0).astype(ml_dtypes.bfloat16)
    ident = np.eye(128, dtype=ml_dtypes.bfloat16)
    betab = beta.reshape(1, C)

    in_maps = []
    for core in range(8):
        b, g = core // 2, core % 2
        sl = slice(g * 512, (g + 1) * 512)
        wq = (w_qkv[0 * C:1 * C][sl] * gamma[None, :]).T.copy()      # [1024, 512]
        wk = (w_qkv[1 * C:2 * C][sl] * gamma[None, :]).T.copy()
        wv = (w_qkv[2 * C:3 * C][sl] * gamma[None, :]).T.copy()
        wo = w_out[:, sl].T.copy()                                    # [512, 1024]
        in_maps.append({
            "x": np.ascontiguousarray(x[b]),
            "wq": wq.reshape(KC, 128, 512).astype(ml_dtypes.bfloat16),
            "wk": wk.reshape(KC, 128, 512).astype(ml_dtypes.bfloat16),
            "wv": wv.reshape(KC, 128, 512).astype(ml_dtypes.bfloat16),
            "wo": wo.reshape(NP, 128, 1024).astype(ml_dtypes.bfloat16),
            "masks": mask,
            "ident": ident,
            "betab": betab,
        })
    res = run_bass_kernel_spmd(nc, in_maps, core_ids=list(range(8)))
    out = np.empty((B, T, C), dtype=np.float32)
    for b in range(B):
        out[b] = res.results[2 * b]["out"] + res.results[2 * b + 1]["out"]
    return out


# revision 37
# speedup vs baseline: 1.1703x; 1.0205x over previous
"""CausalSelfAttention TRN2 kernel: LN + QKV + causal attention + out_proj.

Sharding: 8 cores = 4 batches x 2 head-groups (8 heads each). Each core
computes its batch's LayerNorm, QKV for its heads, causal softmax attention,
and a partial out-projection over its heads' channels; the host sums the two
partials per batch.

Schedule: single woven instruction stream. Per-tt LayerNorm/transpose/QKV
chains act as PE filler units interleaved into the attention J-blocks so the
PE never starves while ACT runs exp. Normalization avoids all HWDGE traffic:
sums ride the PV matmul (ones column in v), the B-half PSUM is moved to
partitions 63..127 by a gpsimd SWDGE DMA, reciprocal factors are broadcast
across partitions with gpsimd partition_broadcast, and out_proj DMAs its
PSUM tiles straight to DRAM.

Per-core layouts (SBUF partition dim first):
  hT   [c, t]   LN(x) transposed via PE, bf16, per-tt tiles
  qT/kT [o, t]  o = head*64+d; head pair (2i,2i+1) shares a 128-partition tile
  v    [t, (h, 65)] bf16, col 64 = ones (PV emits softmax sums as row 64)
  scores sp [tk, 2, tq] per kt tile (both pair halves share one PSUM tile so
  one ACT exp covers them); causal diagonal tiles slice rhs to [offs:512];
  causality inside the first 128 cols via a multiplicative [i>j] mask on DVE.
  out_proj: lhsT = AT [j, t] bf16, rhs = woT [j, o] bf16, PSUM -> DRAM DMA.
"""
import math
import sys
from collections import deque

sys.path.insert(0, "/opt/trn_rl_repo")
sys.path.insert(0, "/opt/trn_rl_repo/concourse")

import numpy as np
import ml_dtypes

import concourse.bass as bass
import concourse.bacc as bacc
import concourse.mybir as mybir
import concourse.tile as tile
from concourse.bass_utils import run_bass_kernel_spmd

T, C, NH, DH = 2048, 1024, 16, 64
HC = 8            # heads per core
NT = T // 128     # 16 t-tiles
KC = C // 128     # 8 contraction tiles
W = 512           # tq block width
NJ = T // W       # 4 q blocks
NP = HC // 2      # 4 head pairs
F32, BF16 = mybir.dt.float32, mybir.dt.bfloat16
AF = mybir.ActivationFunctionType
SUB, MUL = mybir.AluOpType.subtract, mybir.AluOpType.mult

_CACHE = {}
DEBUG = False


def _build(beta_nonzero):
    nc = bacc.Bacc("TRN2", target_bir_lowering=False, debug=False)
    dx = nc.dram_tensor("x", [T, C], F32, kind="ExternalInput")
    dwq = nc.dram_tensor("wq", [KC, 128, 512], BF16, kind="ExternalInput")
    dwk = nc.dram_tensor("wk", [KC, 128, 512], BF16, kind="ExternalInput")
    dwv = nc.dram_tensor("wv", [KC, 128, 512], BF16, kind="ExternalInput")
    dwo = nc.dram_tensor("wo", [NP, 128, 1024], BF16, kind="ExternalInput")
    dmask = nc.dram_tensor("masks", [128, 128], BF16, kind="ExternalInput")
    did = nc.dram_tensor("ident", [128, 128], BF16, kind="ExternalInput")
    dbeta = nc.dram_tensor("betab", [1, C], F32, kind="ExternalInput")
    dout = nc.dram_tensor("out", [T, C], F32, kind="ExternalOutput")
    if DEBUG:
        dbg_q = nc.dram_tensor("dbg_q", [128, NP, T], BF16, kind="ExternalOutput")
        dbg_k = nc.dram_tensor("dbg_k", [128, NP, T], BF16, kind="ExternalOutput")
        dbg_v = nc.dram_tensor("dbg_v", [128, NT, HC, 65], BF16, kind="ExternalOutput")
        dbg_at = nc.dram_tensor("dbg_at", [128, NP, NJ, 512], BF16, kind="ExternalOutput")
        dbg_ua = nc.dram_tensor("dbg_ua", [NJ, NP, 65, 512], F32, kind="ExternalOutput")
        dbg_uh = nc.dram_tensor("dbg_uh", [NJ, NP, 128, 512], F32, kind="ExternalOutput")
        dbg_fac = nc.dram_tensor("dbg_fac", [NJ, NP, 128, 512], F32, kind="ExternalOutput")

    with tile.TileContext(nc) as tc:
        cst = tc.alloc_tile_pool(name="cst", bufs=1)
        ident = cst.tile([128, 128], BF16)
        mask1 = cst.tile([128, 128], BF16)
        eps = cst.tile([128, 1], F32)
        wq_sb = cst.tile([128, KC, 512], BF16)
        wk_sb = cst.tile([128, KC, 512], BF16)
        wv_sb = cst.tile([128, KC, 512], BF16)
        wo_sb = cst.tile([128, NP, 1024], BF16)
        qT = cst.tile([128, NP, T], BF16)
        kT = cst.tile([128, NP, T], BF16)
        v_sb = cst.tile([128, NT, HC, 65], BF16)
        AT = cst.tile([128, NP, NJ, 512], BF16)
        if beta_nonzero:
            beta_sb = cst.tile([128, C], F32)

        ones64 = cst.tile([128, 64], F32)

        xp = tc.alloc_tile_pool(name="xp", bufs=6)
        hp = tc.alloc_tile_pool(name="hp", bufs=3)
        htp = tc.alloc_tile_pool(name="htp", bufs=8)
        stp = tc.alloc_tile_pool(name="stp", bufs=4)
        ptp = tc.alloc_tile_pool(name="ptp", bufs=6)
        uap = tc.alloc_tile_pool(name="uap", bufs=6)
        uhp = tc.alloc_tile_pool(name="uhp", bufs=4)
        facp = tc.alloc_tile_pool(name="facp", bufs=4)
        outp = tc.alloc_tile_pool(name="outp", bufs=4)
        pap = tc.alloc_tile_pool(name="pap", bufs=2, space="PSUM")
        spp = tc.alloc_tile_pool(name="spp", bufs=2, space="PSUM")
        pop = tc.alloc_tile_pool(name="pop", bufs=2, space="PSUM")
        drp = tc.alloc_tile_pool(name="drp", bufs=1, space="DRAM")
        drec = drp.tile([NJ, NP, 2, 512], F32)

        nc.vector.memset(eps[:], 1e-5)
        nc.vector.memset(v_sb[:, :, :, 64:65], 1.0)
        nc.vector.memset(ones64[:], 1.0)

        # ---- DMA issue: sync/HWDGE queue (x0-1, weights, x8-15, wo)
        xts = []
        for tt in range(NT):
            xt = xp.tile([128, C], F32, tag="x", name=f"xt{tt}")
            xts.append(xt)
        # ident on the Pool queue first (first transposes need it ~6us);
        # x0/x1 on sync (earliest arrival), x2-7 on Pool/SWDGE. Writers must
        # appear in allocation order per slot, so keep tt order.
        nc.gpsimd.dma_start(ident[:], did[:])
        nc.sync.dma_start(xts[0][:], dx[0:128, :])
        nc.sync.dma_start(xts[1][:], dx[128:256, :])
        for tt in range(2, 8):
            nc.gpsimd.dma_start(xts[tt][:], dx[tt * 128:(tt + 1) * 128, :])
        for kc in range(KC):
            nc.sync.dma_start(wq_sb[:, kc, :], dwq[kc])
        for kc in range(KC):
            nc.sync.dma_start(wk_sb[:, kc, :], dwk[kc])
        nc.sync.dma_start(mask1[:], dmask[:])
        for kc in range(KC):
            nc.sync.dma_start(wv_sb[:, kc, :], dwv[kc])
        for tt in range(8, NT):
            nc.sync.dma_start(xts[tt][:], dx[tt * 128:(tt + 1) * 128, :])
        for jp in range(NP):
            nc.sync.dma_start(wo_sb[:, jp, :], dwo[jp])
        if beta_nonzero:
            bap = dbeta[0:1, :]
            nc.sync.dma_start(
                beta_sb[:],
                bass.AP(tensor=bap.tensor, offset=bap.offset,
                        ap=[[0, 128], bap.ap[1]]))

        hTs = [None] * NT

        def emit_ln(tt):
            xt = xts[tt]
            stats = stp.tile([128, 2, 6], F32, tag="st")
            xg = xt[:].rearrange("p (g d) -> p g d", g=2)
            for g in range(2):
                nc.vector.bn_stats(stats[:, g, :], xg[:, g, :])
            mv = stp.tile([128, 2], F32, tag="mv")
            nc.vector.bn_aggr(mv[:], stats[:])
            sd = stp.tile([128, 1], F32, tag="sd")
            nc.scalar.activation(sd[:], mv[:, 1:2], AF.Sqrt, bias=eps[:], scale=1.0)
            nc.vector.reciprocal(sd[:], sd[:])
            ht = hp.tile([128, C], BF16, tag="h")
            nc.vector.tensor_scalar(
                out=ht[:], in0=xt[:], scalar1=mv[:, 0:1], scalar2=sd[:],
                op0=SUB, op1=MUL)
            if beta_nonzero:
                nc.vector.tensor_add(ht[:], ht[:], beta_sb[:])
            tp = pap.tile([128, KC, 128], BF16, tag="pa", name="tp")
            for kc in range(KC):
                nc.tensor.transpose(tp[:, kc, :], ht[:, kc * 128:(kc + 1) * 128], ident[:])
            hT = htp.tile([128, KC, 128], BF16, tag="hT", name=f"hT{tt}")
            nc.vector.tensor_copy(hT[:], tp[:])
            hTs[tt] = hT

        def emit_q(tt):
            q4 = pap.tile([128, NP, 128], F32, tag="pa", name="q4")
            for ot in range(NP):
                for kc in range(KC):
                    nc.tensor.matmul(q4[:, ot, :],
                                     wq_sb[:, kc, ot * 128:(ot + 1) * 128],
                                     hTs[tt][:, kc, :],
                                     start=(kc == 0), stop=(kc == KC - 1))
            nc.vector.tensor_copy(qT[:, :, tt * 128:(tt + 1) * 128], q4[:])

        def emit_k(tt):
            k4 = pap.tile([128, NP, 128], F32, tag="pa", name="k4")
            for ot in range(NP):
                for kc in range(KC):
                    nc.tensor.matmul(k4[:, ot, :],
                                     wk_sb[:, kc, ot * 128:(ot + 1) * 128],
                                     hTs[tt][:, kc, :],
                                     start=(kc == 0), stop=(kc == KC - 1))
            nc.vector.tensor_copy(kT[:, :, tt * 128:(tt + 1) * 128], k4[:])

        def emit_v(tt):
            v1 = pap.tile([128, 512], F32, tag="pa", name="v1")
            for kc in range(KC):
                nc.tensor.matmul(v1[:], hTs[tt][:, kc, :], wv_sb[:, kc, :],
                                 start=(kc == 0), stop=(kc == KC - 1))
            nc.vector.tensor_copy(
                v_sb[:, tt, :, 0:64],
                v1[:].rearrange("p (h d) -> p h d", h=HC))

        def emit_out(J, tc4, ob):
            pp = pap.tile([128, 512], F32, tag="pa", name="pp")
            for hp_ in range(NP):
                nc.tensor.matmul(pp[:],
                                 AT[:, hp_, J, tc4 * 128:(tc4 + 1) * 128],
                                 wo_sb[:, hp_, ob * 512:(ob + 1) * 512],
                                 start=(hp_ == 0), stop=(hp_ == NP - 1))
            ot_ = outp.tile([128, 512], F32, tag="ot", name="ot")
            nc.vector.tensor_copy(ot_[:], pp[:])
            t0 = J * 512 + tc4 * 128
            nc.sync.dma_start(dout[t0:t0 + 128, ob * 512:(ob + 1) * 512], ot_[:])

        # ---------------- attention ----------------
        def emit_scores_exp(J, hpair, kt):
            offs = max(0, (kt - 4 * J) * 128)
            sp = spp.tile([128, 2, 512], F32, tag="sp", name="sp")
            for half, base in ((0, 0), (1, 64)):
                nc.tensor.matmul(
                    sp[:, half, offs:512],
                    kT[base:base + 64, hpair, kt * 128:(kt + 1) * 128],
                    qT[base:base + 64, hpair, J * 512 + offs:(J + 1) * 512],
                    start=True, stop=True,
                    tile_position=(base, 0))
            pt = ptp.tile([128, 2, 512], BF16, tag="pt", name="pt")
            if offs == 0:
                nc.scalar.activation(
                    pt[:].rearrange("p g f -> p (g f)"),
                    sp[:].rearrange("p g f -> p (g f)"),
                    AF.Exp, scale=0.125)
            else:
                nc.scalar.activation(pt[:, :, offs:512], sp[:, :, offs:512],
                                     AF.Exp, scale=0.125)
            if kt - 4 * J >= 0:
                r = offs
                for half in range(2):
                    nc.vector.tensor_mul(pt[:, half, r:r + 128],
                                         pt[:, half, r:r + 128],
                                         mask1[:, 0:128])
            return pt, offs

        def emit_pv(kt, nkt, poA, poB, hpair, pt, offs):
            for half, po in ((0, poA), (1, poB)):
                h = 2 * hpair + half
                nc.tensor.matmul(po[:, offs:512], v_sb[:, kt, h, :],
                                 pt[:, half, offs:512],
                                 start=(kt == 0), stop=(kt == nkt - 1))

        def emit_norm1(J, hpair, poA, poB):
            # u copies (DVE, PSUM->SBUF), B-half U rows -> partitions 64..127
            # (gpsimd SWDGE SBUF->SBUF), sums-row broadcasts (gpsimd) into fac.
            uA = uap.tile([65, 512], F32, tag="ua", name="uA")
            nc.vector.tensor_copy(uA[:], poA[:])
            uB = uap.tile([65, 512], F32, tag="ua", name="uB")
            nc.vector.tensor_copy(uB[:], poB[:])
            uhi = uhp.tile([128, 512], F32, tag="uh", name="uhi")
            last = (J == NJ - 1 and hpair == NP - 1)
            if last:
                # Tail: low-latency variant. uhi via sync HWDGE; reciprocal
                # factors via a PE ones-outer-product into the just-freed po
                # PSUM slot (no DRAM roundtrip).
                nc.sync.dma_start(uhi[64:128, :], uB[0:64, :])
                fac = pop.tile([128, 512], F32, tag="po", name="facp_")
                nc.tensor.matmul(fac[0:64, :], ones64[64:65, :], uA[64:65, :],
                                 start=True, stop=True)
                nc.tensor.matmul(fac[64:128, :], ones64[64:65, :], uB[64:65, :],
                                 start=True, stop=True)
                return uA, uhi, fac
            nc.gpsimd.dma_start(uhi[64:128, :], uB[0:64, :])
            # sums rows -> DRAM, then one broadcast load into fac (SBUF src
            # partition dim cannot have stride 0; DRAM src can repeat).
            nc.gpsimd.dma_start(drec[J, hpair, 0:1, :], uA[64:65, :])
            nc.gpsimd.dma_start(drec[J, hpair, 1:2, :], uB[64:65, :])
            fac = facp.tile([128, 512], F32, tag="fac", name="fac")
            rows = drec[J, hpair]
            nc.gpsimd.dma_start(
                fac[:],
                bass.AP(tensor=rows.tensor, offset=rows.offset,
                        ap=[[512, 2], [0, 64], [1, 512]]))
            return uA, uhi, fac

        def emit_norm2(J, hpair, uA, uhi, fac):
            nc.vector.reciprocal(fac[:], fac[:])
            if DEBUG:
                nc.sync.dma_start(dbg_ua[J, hpair], uA[:])
                nc.sync.dma_start(dbg_uh[J, hpair], uhi[:])
                nc.sync.dma_start(dbg_fac[J, hpair], fac[:])
            nc.vector.tensor_mul(AT[0:64, hpair, J, :], uA[0:64, :], fac[0:64, :])
            nc.vector.tensor_mul(AT[64:128, hpair, J, :], uhi[64:128, :],
                                 fac[64:128, :])

        # ---------------- woven emission ----------------
        fq = deque()
        for tt in range(NT):
            fq.append(lambda tt=tt: emit_ln(tt))
            fq.append(lambda tt=tt: emit_q(tt))
            if tt - 2 >= 0:
                fq.append(lambda tt=tt: emit_k(tt - 2))
            if tt - 4 >= 0:
                fq.append(lambda tt=tt: emit_v(tt - 4))
        for tt in range(NT - 2, NT):
            fq.append(lambda tt=tt: emit_k(tt))
        for tt in range(NT - 4, NT):
            fq.append(lambda tt=tt: emit_v(tt))

        emitted = [0]

        def filler(n=1):
            for _ in range(n):
                if fq:
                    fq.popleft()()
                    emitted[0] += 1

        # qkv-unit watermark required before att(J): v(4J+3) must be emitted.
        NEED = [26, 42, 58, 64]
        # per-kt filler modulus per J (sparser late so units survive for the
        # J3 norm tail)
        FRATE = [3, 3, 4, 5]

        pending2 = None          # deferred norm2 args
        pending_out = None       # J whose out chains go to fq next
        prev_pv = None           # PV of the previous (hpair, kt) step
        pending1 = None          # norm1 args of the previous hpair

        def flush_norm2():
            nonlocal pending2, pending_out
            if pending2 is not None:
                emit_norm2(*pending2)
                pending2 = None
                if pending_out is not None:
                    Jo = pending_out
                    for tc4 in range(4):
                        for ob in range(2):
                            fq.append(lambda Jo=Jo, tc4=tc4, ob=ob:
                                      emit_out(Jo, tc4, ob))
                    pending_out = None

        for J in range(NJ):
            nkt = 4 * J + 4
            filler(max(0, NEED[J] - emitted[0]))
            for hpair in range(NP):
                filler(1)
                poA = pop.tile([65, 512], F32, tag="po", name="poA")
                poB = pop.tile([65, 512], F32, tag="po", name="poB")
                for kt in range(nkt):
                    cur = (kt, nkt, poA, poB, hpair) + \
                        emit_scores_exp(J, hpair, kt)
                    if prev_pv is not None:
                        emit_pv(*prev_pv)
                    prev_pv = cur
                    if kt == 0 and pending1 is not None:
                        # previous hpair's last PV just flushed above
                        n1 = emit_norm1(*pending1)
                        pending2 = pending1[:2] + n1
                        pending1 = None
                    if kt % FRATE[J] == FRATE[J] - 1:
                        filler(1)
                # norm2 of the hpair before last: its fac DMA roundtrip has
                # had a full hpair to land, and this hpair's mask muls are
                # already ahead of it in DVE order.
                flush_norm2()
                pending1 = (J, hpair, poA, poB)
            pending_out = J
        emit_pv(*prev_pv)
        n1 = emit_norm1(*pending1)
        emit_norm2(pending1[0], pending1[1], *n1)
        for tc4 in range(4):
            for ob in range(2):
                fq.append(lambda tc4=tc4, ob=ob: emit_out(NJ - 1, tc4, ob))
        filler(len(fq))
        if DEBUG:
            nc.sync.dma_start(dbg_q[:], qT[:])
            nc.sync.dma_start(dbg_k[:], kT[:])
            nc.sync.dma_start(dbg_v[:], v_sb[:])
            nc.sync.dma_start(dbg_at[:], AT[:])

        for p in (drp, pop, spp, pap, outp, facp, uhp, uap, ptp, stp, htp, hp, xp, cst):
            p.release()
    nc.compile()
    return nc


def kernel(x, gamma, beta, w_qkv, w_out):
    x = np.asarray(x, dtype=np.float32)
    gamma = np.asarray(gamma, dtype=np.float32)
    beta = np.asarray(beta, dtype=np.float32)
    w_qkv = np.asarray(w_qkv, dtype=np.float32)
    w_out = np.asarray(w_out, dtype=np.float32)
    B = x.shape[0]
    beta_nonzero = bool(np.any(beta != 0.0))
    key = ("k", beta_nonzero)
    if key not in _CACHE:
        _CACHE[key] = _build(beta_nonzero)
    nc = _CACHE[key]

    i128, j128 = np.indices((128, 128))
    mask = np.where(i128 > j128, 0.0, 1.0).astype(ml_dtypes.bfloat16)
    ident = np.eye(128, dtype=ml_dtypes.bfloat16)
    betab = beta.reshape(1, C)

    in_maps = []
    for core in range(8):
        b, g = core // 2, core % 2
        sl = slice(g * 512, (g + 1) * 512)
        wq = (w_qkv[0 * C:1 * C][sl] * gamma[None, :]).T.copy()      # [1024, 512]
        wk = (w_qkv[1 * C:2 * C][sl] * gamma[None, :]).T.copy()
        wv = (w_qkv[2 * C:3 * C][sl] * gamma[None, :]).T.copy()
        wo = w_out[:, sl].T.copy()                                    # [512, 1024]
        in_maps.append({
            "x": np.ascontiguousarray(x[b]),
            "wq": wq.reshape(KC, 128, 512).astype(ml_dtypes.bfloat16),
            "wk": wk.reshape(KC, 128, 512).astype(ml_dtypes.bfloat16),
            "wv": wv.reshape(KC, 128, 512).astype(ml_dtypes.bfloat16),
            "wo": wo.reshape(NP, 128, 1024).astype(ml_dtypes.bfloat16),
            "masks": mask,
            "ident": ident,
            "betab": betab,
        })
    res = run_bass_kernel_spmd(nc, in_maps, core_ids=list(range(8)))
    out = np.empty((B, T, C), dtype=np.float32)
    for b in range(B):
        out[b] = res.results[2 * b]["out"] + res.results[2 * b + 1]["out"]
    return out
